# revision 1
# baseline (speedup 1.0000x reference)
"""Causal dot-product attention for Trainium2 (Bass/Tile), 8-core SPMD.

Problem: B=32, T=2048, D=64 fp32.  reference:
    O = softmax(mask(Q K^T / sqrt(D))) V      (causal mask, per batch)

Sharding: pure batch parallelism - 4 batches per NeuronCore, no collectives.

Per-core algorithm (flash-style; no online rescale needed: scores ~ N(0,1),
so exp() is computed directly with a constant stability shift that cancels
in the softmax):

  S^T layout (= K Q^T) so the PV contraction (over key positions) lands on
  the partition dim and the softmax sums ride along for free as a
  ones-column of V (row 64 of the transposed PV accumulator).

  The S^T contraction dim is only D=64, so pairs of key chunks are packed
  into the two 64-row halves of the PE array (tile_position row packing,
  auto-derived from operand base partitions) and run concurrently - the
  concurrent pair MUST target different PSUM banks (same-bank concurrent
  PE writes are a hard fault: NRT_EXEC_UNIT_UNRECOVERABLE).  Host-side
  prep supplies Q^T duplicated into both partition halves and K^T with
  even/odd chunks interleaved, plus the ones-augmented V, so the kernel
  performs no input transposes.

  Per batch (16 key chunks of 128, 4 query tiles of 512):
    for each q-tile i, key-chunk pair u (diagonal pairs first, so the
    mask latency hides under the off-diagonal pipeline):
      S^T pair -> one PSUM [128,1024] tile (half-width N=256 for the
      outer diagonal pair), one ACT exp(s/8 - 2) pass PSUM->SBUF,
      DVE multiplies by precomputed 0/1 masks zero the causal triangles
      (small regions only), PV accumulates O^T [65, 512] (start flag on
      the first full-width matmul initializes the whole bank).
    epilogue per q-tile: DVE copy O^T to SBUF, 4 PE transposes back to
    [q, 65], DVE reciprocal of the sums row, scale, DMA out.

Matmuls run in float32r (fp32 bits, PE "replicated" mode, fp32 PSUM
accumulation; ~2 cyc/col, ~1e-4 relative rounding).  bf16 PV and a PE
"heater" were measured slower and are kept behind env flags (off).
"""

import os

# Standard recovery knob: reset NeuronCores at runtime init (harmless on a
# healthy device, helps if a previous run left cores wedged). Set before
# backend init; a no-op if the caller already configured it.
os.environ.setdefault("NEURON_RT_RESET_CORES", "1")

import ml_dtypes
import numpy as np

import concourse.bacc as bacc
import concourse.mybir as mybir
import concourse.tile as tile
from concourse.masks import make_identity
from concourse.bass_utils import run_bass_kernel_spmd

B, T, D = 32, 2048, 64
NCORES = 8
BL = B // NCORES            # batches per core
P = 128                     # partitions / key-chunk size
NCH = T // P                # key chunks per batch (16)
QW = 512                    # query-tile width
NQT = T // QW               # query tiles per batch (4)
SCALE = 1.0 / np.sqrt(D)    # 0.125
EBIAS = -2.0                # stability shift inside exp(); cancels in softmax

F32 = mybir.dt.float32
F32R = mybir.dt.float32r
BF16 = mybir.dt.bfloat16

HALF_DIAG = os.environ.get("ATTN_HALF_DIAG", "1") == "1"
# PV (attention-weights x values) in bf16: P~ and V rounding errors average
# out across the softmax; S^T stays fp32r for score precision.
BF16_PV = os.environ.get("ATTN_BF16_PV", "0") == "1"
PVDT = BF16 if BF16_PV else F32R
HEATER = os.environ.get("ATTN_HEATER", "0") == "1"
TRP_F32R = os.environ.get("ATTN_TRP_F32R", "0") == "1"
# epilogue transpose as a regular fp32r matmul (osb.T @ I) instead of the
# 2-pass transpose-mode instruction
MM_TRANSPOSE = os.environ.get("ATTN_MM_TRANSPOSE", "0") == "1"
# pre-warm burst during the head DMA stall (see below)
PREWARM = os.environ.get("ATTN_PREWARM", "1") == "1"
PREWARM_N = int(os.environ.get("ATTN_PREWARM_N", "12"))
# sparse in-stream heater: tiny bf16 matmul every 2nd pair, accumulated into
# unused partitions (96+) of the live O^T accumulator bank
SPARSE_HEAT = os.environ.get("ATTN_SPARSE_HEAT", "0") == "1"


def build_nc():
    from contextlib import ExitStack

    nc = bacc.Bacc()
    # host-prepped inputs:
    #   q2: Q^T duplicated into both partition halves      [BL, 128, T]
    #   k2: K^T, even chunks rows 0:64, odd rows 64:128    [BL, 128, T/2]
    #   v:  V with ones column                             [BL, T, D+1]
    q2_d = nc.dram_tensor("q2", [BL, P, T], F32, kind="ExternalInput")
    k2_d = nc.dram_tensor("k2", [BL, P, T // 2], F32, kind="ExternalInput")
    v_d = nc.dram_tensor("v", [BL, T, D + 1], PVDT, kind="ExternalInput")
    o_d = nc.dram_tensor("o", [BL, T, D], F32, kind="ExternalOutput")

    with tile.TileContext(nc) as tc, ExitStack() as ctx:
        singles = ctx.enter_context(tc.tile_pool(name="singles", bufs=1))
        wpool = ctx.enter_context(tc.tile_pool(name="wts", bufs=4))
        pepool = ctx.enter_context(tc.tile_pool(name="pexp", bufs=8))
        osb_pool = ctx.enter_context(tc.tile_pool(name="osb", bufs=3))
        oout_pool = ctx.enter_context(tc.tile_pool(name="oout", bufs=3))
        rec_pool = ctx.enter_context(tc.tile_pool(name="rec", bufs=8))
        st_ps = ctx.enter_context(
            tc.tile_pool(name="stps", bufs=2 if HEATER else 3, space="PSUM")
        )
        ht_ps = ctx.enter_context(tc.tile_pool(name="htps", bufs=1, space="PSUM"))
        ot_ps = ctx.enter_context(tc.tile_pool(name="otps", bufs=2, space="PSUM"))

        ident = singles.tile([P, P], F32)
        make_identity(nc, ident)
        if TRP_F32R or MM_TRANSPOSE:
            identr = singles.tile([P, P], F32R)
            nc.vector.tensor_copy(out=identr, in_=ident)
        else:
            identr = ident
        ebias = singles.tile([P, 1], F32)
        nc.vector.memset(ebias, EBIAS)
        # precomputed 0/1 causal masks, applied by DVE multiplies:
        #   tri0: keep where f >= p      (the diagonal 128-triangle)
        #   msk1: keep where f >= 128+p  (one full masked chunk + triangle)
        tri0 = singles.tile([P, P], F32)
        nc.vector.memset(tri0, 1.0)
        nc.gpsimd.affine_select(
            out=tri0, in_=tri0, compare_op=mybir.AluOpType.is_ge, fill=0.0,
            base=0, channel_multiplier=-1, pattern=[[1, P]],
        )
        msk1 = singles.tile([P, 2 * P], F32)
        nc.vector.memset(msk1, 1.0)
        nc.gpsimd.affine_select(
            out=msk1, in_=msk1, compare_op=mybir.AluOpType.is_ge, fill=0.0,
            base=-P, channel_multiplier=-1, pattern=[[1, 2 * P]],
        )

        if HEATER:
            hb = singles.tile([1, 4], BF16)
            nc.vector.memset(hb, 1.0)
            heat = ht_ps.tile([P, 4], F32, tag="heat")

        if SPARSE_HEAT:
            shb = singles.tile([1, 4], BF16)
            nc.vector.memset(shb, 1.0)

        if PREWARM:
            # dense bf16 matmul burst on dummy data, scheduled during the
            # initial input-DMA stall (no data deps): holds the PE busy for
            # >3.4us so the HAM clock gate opens to 2.4 GHz before the real
            # fp32r stream starts. Uses an "ot" pool slot (released before
            # the first accumulator is needed) -> no extra PSUM bank.
            wsrc = singles.tile([P, QW], BF16)
            nc.vector.memset(wsrc, 0.5)
            wps = ot_ps.tile([P, QW], F32, tag="ot", name="warm")
            for _ in range(PREWARM_N):
                nc.tensor.matmul(
                    out=wps, lhsT=wsrc[:, 0:P], rhs=wsrc,
                    start=True, stop=True,
                )

        def heater():
            # tiny bf16 matmul: keeps the PE HAM activity monitor warm so
            # the fp32r matmuls run at 2.4 GHz instead of the cold 1.2 GHz
            if HEATER:
                nc.tensor.matmul(
                    out=heat[0:1, 0:4], lhsT=hb[0:1, 0:1], rhs=hb[0:1, 0:4],
                    start=True, stop=True,
                )

        def load_batch(b):
            qt = wpool.tile([P, T], F32R, tag="qt", name=f"qt{b}")
            nc.sync.dma_start(out=qt, in_=q2_d[b].bitcast(F32R))
            kt = wpool.tile([P, T // 2], F32R, tag="kt", name=f"kt{b}")
            nc.sync.dma_start(out=kt, in_=k2_d[b].bitcast(F32R))
            vv = wpool.tile([P, NCH, D + 1], PVDT, tag="vv", name=f"vv{b}")
            vsrc = v_d[b].rearrange("(c p) d -> p c d", p=P)
            if not BF16_PV:
                vsrc = vsrc.bitcast(F32R)
            nc.sync.dma_start(out=vv, in_=vsrc)
            return [qt], [kt], [vv]

        def compute_qtile(b, i, qts, kts, vvs):
            otp = ot_ps.tile([P, QW], F32, tag="ot", name=f"ot{b}_{i}")
            # process pairs diagonal-first so the GPSIMD mask latency
            # hides under the off-diagonal pipeline; the full-width pair
            # leads so its start=True matmul initializes the whole
            # accumulator bank
            # lead with a maskless off-diagonal pair (shortest chain to the
            # start=True PV), then the diagonal pairs so their mask latency
            # still hides under the remaining off-diagonal pipeline
            if i == 0:
                order = [0, 1]
            else:
                order = [0, 2 * i, 2 * i + 1] + list(range(1, 2 * i))
            last_u = order[-1]
            for oidx, u in enumerate(order):
                heater()
                if SPARSE_HEAT and oidx >= 1 and oidx % 2 == 1:
                    # bf16 blip for the HAM activity monitor; accumulates
                    # into never-read cells (partitions 96, cols 0:4) of the
                    # already-started accumulator bank
                    # self-contained 1-partition accumulation group on
                    # partition 96 (outside the otp group's partitions 0-64)
                    nc.tensor.matmul(
                        out=otp[96:97, 0:4],
                        lhsT=shb[0:1, 0:1],
                        rhs=shb[0:1, 0:4],
                        start=True,
                        stop=True,
                        tile_position=(0, 96),
                        skip_group_check=True,
                    )
                start = oidx == 0
                stop = u == last_u
                stp = st_ps.tile(
                    [P, 2 * QW], F32, tag="st", name=f"st{b}_{i}_{u}"
                )
                pexp = pepool.tile(
                    [P, 2 * QW], PVDT, tag="pe", name=f"pe{b}_{i}_{u}"
                )
                if HALF_DIAG and u == 2 * i + 1:
                    # outer diagonal pair: only q_local in [256, 512)
                    # can be unmasked -> compute half width (N=256)
                    for h in range(2):
                        # concurrent row-packed matmuls must target
                        # DIFFERENT PSUM banks -> bank h, cols [0,256)
                        nc.tensor.matmul(
                            out=stp[:, h * QW : h * QW + 256],
                            lhsT=kts[0][h * D : (h + 1) * D, u * P : (u + 1) * P],
                            rhs=qts[0][h * D : (h + 1) * D, i * QW + 256 : (i + 1) * QW],
                            start=True,
                            stop=True,
                        )
                    for h in range(2):
                        nc.scalar.activation(
                            out=pexp[:, h * 256 : (h + 1) * 256],
                            in_=stp[:, h * QW : h * QW + 256],
                            func=mybir.ActivationFunctionType.Exp,
                            bias=ebias,
                            scale=SCALE,
                        )
                    # chunk 4i+2: cols 0:256 <-> q_local 256+f, kp 256+p
                    nc.vector.tensor_mul(
                        out=pexp[:, 0:P], in0=pexp[:, 0:P], in1=tri0
                    )
                    # chunk 4i+3: cols 256:512 <-> q_local 256+f, kp 384+p
                    nc.vector.tensor_mul(
                        out=pexp[:, 256:QW], in0=pexp[:, 256:QW], in1=msk1
                    )
                    for h in range(2):
                        nc.tensor.matmul(
                            out=otp[0 : D + 1, 256:QW],
                            lhsT=vvs[0][:, 2 * u + h, :],
                            rhs=pexp[:, h * 256 : (h + 1) * 256],
                            start=start and h == 0,
                            stop=stop and h == 1,
                        )
                    continue
                # full-width pair
                for h in range(2):
                    nc.tensor.matmul(
                        out=stp[:, h * QW : (h + 1) * QW],
                        lhsT=kts[0][h * D : (h + 1) * D, u * P : (u + 1) * P],
                        rhs=qts[0][h * D : (h + 1) * D, i * QW : (i + 1) * QW],
                        start=True,
                        stop=True,
                    )
                nc.scalar.activation(
                    out=pexp,
                    in_=stp,
                    func=mybir.ActivationFunctionType.Exp,
                    bias=ebias,
                    scale=SCALE,
                )
                if u == 2 * i:
                    # inner diagonal pair: chunk 4i triangle at cols 0:128,
                    # chunk 4i+1 masked+triangle at cols 512:768
                    nc.vector.tensor_mul(
                        out=pexp[:, 0:P], in0=pexp[:, 0:P], in1=tri0
                    )
                    nc.vector.tensor_mul(
                        out=pexp[:, QW : QW + 2 * P],
                        in0=pexp[:, QW : QW + 2 * P],
                        in1=msk1,
                    )
                for h in range(2):
                    # chunk 4i+1 is fully masked below q_local=128: trim its
                    # dead first 128 columns from the PV stream (N=384)
                    lo = P if (u == 2 * i and h == 1) else 0
                    nc.tensor.matmul(
                        out=otp[0 : D + 1, :] if lo == 0 else otp[0 : D + 1, lo:QW],
                        lhsT=vvs[0][:, 2 * u + h, :],
                        rhs=pexp[:, h * QW + lo : (h + 1) * QW],
                        start=start and h == 0,
                        stop=stop and h == 1,
                    )
            # epilogue: O^T [65, 512] -> O [512, 64] / sums
            osb = osb_pool.tile(
                [D + 1, QW], F32R if (TRP_F32R or MM_TRANSPOSE) else F32,
                tag="osb", name=f"osb{b}_{i}",
            )
            nc.vector.tensor_copy(out=osb, in_=otp[0 : D + 1, :])
            # N=66 (even) for the fp32r transpose-matmul; col 65 is zero
            tw = (D + 2) if MM_TRANSPOSE else (D + 1)
            trp = ot_ps.tile(
                [P, 4 * tw], F32R if TRP_F32R else F32,
                tag="ot", name=f"trp{b}_{i}",
            )
            assert not (TRP_F32R and MM_TRANSPOSE)
            oout = oout_pool.tile([P, 4, D], F32, tag="oo", name=f"oo{b}_{i}")
            for m in range(4):
                if MM_TRANSPOSE:
                    nc.tensor.matmul(
                        out=trp[:, m * tw : m * tw + D + 2],
                        lhsT=osb[:, m * P : (m + 1) * P],
                        rhs=identr[0 : D + 1, 0 : D + 2],
                        start=True,
                        stop=True,
                    )
                else:
                    nc.tensor.transpose(
                        out=trp[:, m * tw : m * tw + D + 1],
                        in_=osb[:, m * P : (m + 1) * P],
                        identity=identr[0 : D + 1, 0 : D + 1],
                    )
                rec = rec_pool.tile([P, 1], F32, tag="rec", name=f"rec{b}_{i}_{m}")
                nc.vector.reciprocal(
                    out=rec, in_=trp[:, m * tw + D : m * tw + D + 1]
                )
                nc.vector.tensor_scalar_mul(
                    out=oout[:, m, :],
                    in0=trp[:, m * tw : m * tw + D],
                    scalar1=rec,
                )
            nc.sync.dma_start(
                out=o_d[b, i * QW : (i + 1) * QW, :].rearrange(
                    "(m p) d -> p m d", p=P
                ),
                in_=oout,
            )

        for b in range(BL):
            qts, kts, vvs = load_batch(b)
            for i in range(NQT):
                compute_qtile(b, i, qts, kts, vvs)

    return nc


_NC_CACHE = None


def _get_nc():
    global _NC_CACHE
    if _NC_CACHE is None:
        nc = build_nc()
        nc.finalize()
        _NC_CACHE = nc
    return _NC_CACHE


def prep_inputs(queries, keys, values):
    """Host-side shard + layout prep (numpy only)."""
    q = np.asarray(queries, dtype=np.float32)
    k = np.asarray(keys, dtype=np.float32)
    v = np.asarray(values, dtype=np.float32)
    assert q.shape == (B, T, D), q.shape
    qT = q.transpose(0, 2, 1)                                  # [B, 64, T]
    q2 = np.concatenate([qT, qT], axis=1)                      # [B, 128, T]
    kT = k.transpose(0, 2, 1).reshape(B, D, NCH, P)            # [B, 64, 16, 128]
    k2 = np.concatenate(
        [
            kT[:, :, 0::2, :].reshape(B, D, T // 2),
            kT[:, :, 1::2, :].reshape(B, D, T // 2),
        ],
        axis=1,
    )                                                          # [B, 128, T/2]
    va = np.concatenate([v, np.ones((B, T, 1), np.float32)], axis=-1)
    if BF16_PV:
        va = va.astype(ml_dtypes.bfloat16)
    q2 = np.ascontiguousarray(q2)
    k2 = np.ascontiguousarray(k2)
    va = np.ascontiguousarray(va)
    return [
        {
            "q2": q2[c * BL : (c + 1) * BL],
            "k2": k2[c * BL : (c + 1) * BL],
            "v": va[c * BL : (c + 1) * BL],
        }
        for c in range(NCORES)
    ]


def run(queries, keys, values, trace=False):
    nc = _get_nc()
    core_ids = list(range(NCORES))
    in_maps = prep_inputs(queries, keys, values)
    try:
        res = run_bass_kernel_spmd(nc, in_maps, core_ids, trace=trace)
    except Exception:
        # transient NRT_EXEC_UNIT_UNRECOVERABLE has been observed once in
        # ~30 runs; a straight retry recovers
        res = run_bass_kernel_spmd(nc, in_maps, core_ids, trace=trace)
    out = np.concatenate([res.results[c]["o"] for c in core_ids], axis=0)
    return out.astype(np.float32), res


def kernel(queries, keys, values):
    out, _ = run(queries, keys, values, trace=False)
    return out



# revision 6
# speedup vs baseline: 1.1831x; 1.1831x over previous
"""Causal dot-product attention for Trainium2 (Bass/Tile), 8-core SPMD.

Problem: B=32, T=2048, D=64 fp32.  reference:
    O = softmax(mask(Q K^T / sqrt(D))) V      (causal mask, per batch)

Sharding: pure batch parallelism - 4 batches per NeuronCore, no collectives.

Per-core algorithm (flash-style; no online rescale needed: scores ~ N(0,1),
so exp() is computed directly with a constant stability shift that cancels
in the softmax):

  S^T layout (= K Q^T) so the PV contraction (over key positions) lands on
  the partition dim and the softmax sums ride along for free as a
  ones-column of V (row 64 of the transposed PV accumulator).

  The S^T contraction dim is only D=64, so pairs of key chunks are packed
  into the two 64-row halves of the PE array (tile_position row packing,
  auto-derived from operand base partitions) and run concurrently - the
  concurrent pair MUST target different PSUM banks (same-bank concurrent
  PE writes are a hard fault: NRT_EXEC_UNIT_UNRECOVERABLE).  Host-side
  prep supplies Q^T duplicated into both partition halves and K^T with
  even/odd chunks interleaved, plus the ones-augmented V, so the kernel
  performs no input transposes.

  Per batch (16 key chunks of 128, 4 query tiles of 512):
    for each q-tile i, key-chunk pair u (diagonal pairs first, so the
    mask latency hides under the off-diagonal pipeline):
      S^T pair -> one PSUM [128,1024] tile (half-width N=256 for the
      outer diagonal pair), one ACT exp(s/8 - 2) pass PSUM->SBUF,
      DVE multiplies by precomputed 0/1 masks zero the causal triangles
      (small regions only), PV accumulates O^T [65, 512] (start flag on
      the first full-width matmul initializes the whole bank).
    epilogue per q-tile: DVE copy O^T to SBUF, 4 PE transposes back to
    [q, 65], DVE reciprocal of the sums row, scale, DMA out.

Matmuls run in float32r (fp32 bits, PE "replicated" mode, fp32 PSUM
accumulation; ~2 cyc/col, ~1e-4 relative rounding).  bf16 PV and a PE
"heater" were measured slower and are kept behind env flags (off).
"""

import os

# Standard recovery knob: reset NeuronCores at runtime init (harmless on a
# healthy device, helps if a previous run left cores wedged). Set before
# backend init; a no-op if the caller already configured it.
os.environ.setdefault("NEURON_RT_RESET_CORES", "1")

import ml_dtypes
import numpy as np

import concourse.bacc as bacc
import concourse.mybir as mybir
import concourse.tile as tile
from concourse.masks import make_identity
from concourse.bass_utils import run_bass_kernel_spmd

B, T, D = 32, 2048, 64
NCORES = 8
BL = B // NCORES            # batches per core
P = 128                     # partitions / key-chunk size
NCH = T // P                # key chunks per batch (16)
QW = 512                    # query-tile width
NQT = T // QW               # query tiles per batch (4)
SCALE = 1.0 / np.sqrt(D)    # 0.125
EBIAS = -2.0                # stability shift inside exp(); cancels in softmax

F32 = mybir.dt.float32
F32R = mybir.dt.float32r
BF16 = mybir.dt.bfloat16

HALF_DIAG = os.environ.get("ATTN_HALF_DIAG", "1") == "1"
# All matmuls in bf16: FWL (fast weight load) halves LDWEIGHTS, the dense
# bf16 stream keeps the HAM clock gate open (fp32r mode left the PE at
# 1.2 GHz for ~60% of the kernel), and input DMA halves.  PSUM accumulation
# and the softmax epilogue stay fp32.
BF16_MM = os.environ.get("ATTN_BF16_MM", "1") == "1"
MMDT = BF16 if BF16_MM else F32R
BF16_PV = os.environ.get("ATTN_BF16_PV", "0") == "1" or BF16_MM
PVDT = BF16 if BF16_PV else F32R
HEATER = os.environ.get("ATTN_HEATER", "0") == "1"
TRP_F32R = os.environ.get("ATTN_TRP_F32R", "0") == "1"
# epilogue transpose as a regular fp32r matmul (osb.T @ I) instead of the
# 2-pass transpose-mode instruction
MM_TRANSPOSE = os.environ.get("ATTN_MM_TRANSPOSE", "0") == "1"
# pre-warm burst during the head DMA stall (see below)
PREWARM = os.environ.get("ATTN_PREWARM", "1") == "1"
PREWARM_N = int(os.environ.get("ATTN_PREWARM_N", "12"))
# sparse in-stream heater: tiny bf16 matmul every 2nd pair, accumulated into
# unused partitions (96+) of the live O^T accumulator bank
SPARSE_HEAT = os.environ.get("ATTN_SPARSE_HEAT", "0") == "1"


def build_nc():
    from contextlib import ExitStack

    nc = bacc.Bacc()
    # host-prepped inputs:
    #   q2: Q^T duplicated into both partition halves      [BL, 128, T]
    #   k2: K^T, even chunks rows 0:64, odd rows 64:128    [BL, 128, T/2]
    #   v:  V with ones column                             [BL, T, D+1]
    q2_d = nc.dram_tensor("q2", [BL, P, T], MMDT, kind="ExternalInput")
    k2_d = nc.dram_tensor("k2", [BL, P, T // 2], MMDT, kind="ExternalInput")
    v_d = nc.dram_tensor("v", [BL, T, D + 1], PVDT, kind="ExternalInput")
    o_d = nc.dram_tensor("o", [BL, T, D], F32, kind="ExternalOutput")

    with tile.TileContext(nc) as tc, ExitStack() as ctx:
        singles = ctx.enter_context(tc.tile_pool(name="singles", bufs=1))
        wpool = ctx.enter_context(tc.tile_pool(name="wts", bufs=4))
        pepool = ctx.enter_context(tc.tile_pool(name="pexp", bufs=8))
        osb_pool = ctx.enter_context(tc.tile_pool(name="osb", bufs=3))
        oout_pool = ctx.enter_context(tc.tile_pool(name="oout", bufs=3))
        rec_pool = ctx.enter_context(tc.tile_pool(name="rec", bufs=8))
        st_ps = ctx.enter_context(
            tc.tile_pool(name="stps", bufs=2 if HEATER else 3, space="PSUM")
        )
        ht_ps = ctx.enter_context(tc.tile_pool(name="htps", bufs=1, space="PSUM"))
        ot_ps = ctx.enter_context(tc.tile_pool(name="otps", bufs=2, space="PSUM"))

        ident = singles.tile([P, P], F32)
        make_identity(nc, ident)
        if TRP_F32R or MM_TRANSPOSE:
            identr = singles.tile([P, P], F32R)
            nc.vector.tensor_copy(out=identr, in_=ident)
        else:
            identr = ident
        ebias = singles.tile([P, 1], F32)
        nc.vector.memset(ebias, EBIAS)
        # precomputed 0/1 causal masks, applied by DVE multiplies:
        #   tri0: keep where f >= p      (the diagonal 128-triangle)
        #   msk1: keep where f >= 128+p  (one full masked chunk + triangle)
        tri0f = singles.tile([P, P], F32)
        nc.vector.memset(tri0f, 1.0)
        nc.gpsimd.affine_select(
            out=tri0f, in_=tri0f, compare_op=mybir.AluOpType.is_ge, fill=0.0,
            base=0, channel_multiplier=-1, pattern=[[1, P]],
        )
        msk1f = singles.tile([P, 2 * P], F32)
        nc.vector.memset(msk1f, 1.0)
        nc.gpsimd.affine_select(
            out=msk1f, in_=msk1f, compare_op=mybir.AluOpType.is_ge, fill=0.0,
            base=-P, channel_multiplier=-1, pattern=[[1, 2 * P]],
        )
        if PVDT != F32:
            tri0 = singles.tile([P, P], PVDT)
            nc.vector.tensor_copy(out=tri0, in_=tri0f)
            msk1 = singles.tile([P, 2 * P], PVDT)
            nc.vector.tensor_copy(out=msk1, in_=msk1f)
        else:
            tri0, msk1 = tri0f, msk1f

        if HEATER:
            hb = singles.tile([1, 4], BF16)
            nc.vector.memset(hb, 1.0)
            heat = ht_ps.tile([P, 4], F32, tag="heat")

        if SPARSE_HEAT:
            shb = singles.tile([1, 4], BF16)
            nc.vector.memset(shb, 1.0)

        if PREWARM:
            # dense bf16 matmul burst on dummy data, scheduled during the
            # initial input-DMA stall (no data deps): holds the PE busy for
            # >3.4us so the HAM clock gate opens to 2.4 GHz before the real
            # fp32r stream starts. Uses an "ot" pool slot (released before
            # the first accumulator is needed) -> no extra PSUM bank.
            wsrc = singles.tile([P, QW], BF16)
            nc.vector.memset(wsrc, 0.5)
            wps = ot_ps.tile([P, QW], F32, tag="ot", name="warm")
            for _ in range(PREWARM_N):
                nc.tensor.matmul(
                    out=wps, lhsT=wsrc[:, 0:P], rhs=wsrc,
                    start=True, stop=True,
                )

        def heater():
            # tiny bf16 matmul: keeps the PE HAM activity monitor warm so
            # the fp32r matmuls run at 2.4 GHz instead of the cold 1.2 GHz
            if HEATER:
                nc.tensor.matmul(
                    out=heat[0:1, 0:4], lhsT=hb[0:1, 0:1], rhs=hb[0:1, 0:4],
                    start=True, stop=True,
                )

        def load_batch(b):
            qt = wpool.tile([P, T], MMDT, tag="qt", name=f"qt{b}")
            qsrc = q2_d[b] if BF16_MM else q2_d[b].bitcast(F32R)
            nc.sync.dma_start(out=qt, in_=qsrc)
            kt = wpool.tile([P, T // 2], MMDT, tag="kt", name=f"kt{b}")
            ksrc = k2_d[b] if BF16_MM else k2_d[b].bitcast(F32R)
            nc.sync.dma_start(out=kt, in_=ksrc)
            vv = wpool.tile([P, NCH, D + 1], PVDT, tag="vv", name=f"vv{b}")
            vsrc = v_d[b].rearrange("(c p) d -> p c d", p=P)
            if not BF16_PV:
                vsrc = vsrc.bitcast(F32R)
            nc.sync.dma_start(out=vv, in_=vsrc)
            return [qt], [kt], [vv]

        def compute_qtile(b, i, qts, kts, vvs):
            otp = ot_ps.tile([P, QW], F32, tag="ot", name=f"ot{b}_{i}")
            # process pairs diagonal-first so the GPSIMD mask latency
            # hides under the off-diagonal pipeline; the full-width pair
            # leads so its start=True matmul initializes the whole
            # accumulator bank
            # lead with a maskless off-diagonal pair (shortest chain to the
            # start=True PV), then the diagonal pairs so their mask latency
            # still hides under the remaining off-diagonal pipeline
            if i == 0:
                order = [0, 1]
            else:
                order = [0, 2 * i, 2 * i + 1] + list(range(1, 2 * i))
            last_u = order[-1]
            for oidx, u in enumerate(order):
                heater()
                if SPARSE_HEAT and oidx >= 1 and oidx % 2 == 1:
                    # bf16 blip for the HAM activity monitor; accumulates
                    # into never-read cells (partitions 96, cols 0:4) of the
                    # already-started accumulator bank
                    # self-contained 1-partition accumulation group on
                    # partition 96 (outside the otp group's partitions 0-64)
                    nc.tensor.matmul(
                        out=otp[96:97, 0:4],
                        lhsT=shb[0:1, 0:1],
                        rhs=shb[0:1, 0:4],
                        start=True,
                        stop=True,
                        tile_position=(0, 96),
                        skip_group_check=True,
                    )
                start = oidx == 0
                stop = u == last_u
                stp = st_ps.tile(
                    [P, 2 * QW], F32, tag="st", name=f"st{b}_{i}_{u}"
                )
                pexp = pepool.tile(
                    [P, 2 * QW], PVDT, tag="pe", name=f"pe{b}_{i}_{u}"
                )
                if HALF_DIAG and u == 2 * i + 1:
                    # outer diagonal pair: only q_local in [256, 512)
                    # can be unmasked -> compute half width (N=256)
                    for h in range(2):
                        # concurrent row-packed matmuls must target
                        # DIFFERENT PSUM banks -> bank h, cols [0,256)
                        nc.tensor.matmul(
                            out=stp[:, h * QW : h * QW + 256],
                            lhsT=kts[0][h * D : (h + 1) * D, u * P : (u + 1) * P],
                            rhs=qts[0][h * D : (h + 1) * D, i * QW + 256 : (i + 1) * QW],
                            start=True,
                            stop=True,
                        )
                    for h in range(2):
                        nc.scalar.activation(
                            out=pexp[:, h * 256 : (h + 1) * 256],
                            in_=stp[:, h * QW : h * QW + 256],
                            func=mybir.ActivationFunctionType.Exp,
                            bias=ebias,
                            scale=SCALE,
                        )
                    # chunk 4i+2: cols 0:256 <-> q_local 256+f, kp 256+p
                    nc.vector.tensor_mul(
                        out=pexp[:, 0:P], in0=pexp[:, 0:P], in1=tri0
                    )
                    # chunk 4i+3: cols 256:512 <-> q_local 256+f, kp 384+p
                    nc.vector.tensor_mul(
                        out=pexp[:, 256:QW], in0=pexp[:, 256:QW], in1=msk1
                    )
                    for h in range(2):
                        nc.tensor.matmul(
                            out=otp[0 : D + 1, 256:QW],
                            lhsT=vvs[0][:, 2 * u + h, :],
                            rhs=pexp[:, h * 256 : (h + 1) * 256],
                            start=start and h == 0,
                            stop=stop and h == 1,
                        )
                    continue
                # full-width pair
                for h in range(2):
                    nc.tensor.matmul(
                        out=stp[:, h * QW : (h + 1) * QW],
                        lhsT=kts[0][h * D : (h + 1) * D, u * P : (u + 1) * P],
                        rhs=qts[0][h * D : (h + 1) * D, i * QW : (i + 1) * QW],
                        start=True,
                        stop=True,
                    )
                nc.scalar.activation(
                    out=pexp,
                    in_=stp,
                    func=mybir.ActivationFunctionType.Exp,
                    bias=ebias,
                    scale=SCALE,
                )
                if u == 2 * i:
                    # inner diagonal pair: chunk 4i triangle at cols 0:128,
                    # chunk 4i+1 masked+triangle at cols 512:768
                    nc.vector.tensor_mul(
                        out=pexp[:, 0:P], in0=pexp[:, 0:P], in1=tri0
                    )
                    nc.vector.tensor_mul(
                        out=pexp[:, QW : QW + 2 * P],
                        in0=pexp[:, QW : QW + 2 * P],
                        in1=msk1,
                    )
                for h in range(2):
                    # chunk 4i+1 is fully masked below q_local=128: trim its
                    # dead first 128 columns from the PV stream (N=384)
                    lo = P if (u == 2 * i and h == 1) else 0
                    nc.tensor.matmul(
                        out=otp[0 : D + 1, :] if lo == 0 else otp[0 : D + 1, lo:QW],
                        lhsT=vvs[0][:, 2 * u + h, :],
                        rhs=pexp[:, h * QW + lo : (h + 1) * QW],
                        start=start and h == 0,
                        stop=stop and h == 1,
                    )
            # epilogue: O^T [65, 512] -> O [512, 64] / sums
            osb = osb_pool.tile(
                [D + 1, QW], F32R if (TRP_F32R or MM_TRANSPOSE) else F32,
                tag="osb", name=f"osb{b}_{i}",
            )
            nc.vector.tensor_copy(out=osb, in_=otp[0 : D + 1, :])
            # N=66 (even) for the fp32r transpose-matmul; col 65 is zero
            tw = (D + 2) if MM_TRANSPOSE else (D + 1)
            trp = ot_ps.tile(
                [P, 4 * tw], F32R if TRP_F32R else F32,
                tag="ot", name=f"trp{b}_{i}",
            )
            assert not (TRP_F32R and MM_TRANSPOSE)
            oout = oout_pool.tile([P, 4, D], F32, tag="oo", name=f"oo{b}_{i}")
            for m in range(4):
                if MM_TRANSPOSE:
                    nc.tensor.matmul(
                        out=trp[:, m * tw : m * tw + D + 2],
                        lhsT=osb[:, m * P : (m + 1) * P],
                        rhs=identr[0 : D + 1, 0 : D + 2],
                        start=True,
                        stop=True,
                    )
                else:
                    nc.tensor.transpose(
                        out=trp[:, m * tw : m * tw + D + 1],
                        in_=osb[:, m * P : (m + 1) * P],
                        identity=identr[0 : D + 1, 0 : D + 1],
                    )
                rec = rec_pool.tile([P, 1], F32, tag="rec", name=f"rec{b}_{i}_{m}")
                nc.vector.reciprocal(
                    out=rec, in_=trp[:, m * tw + D : m * tw + D + 1]
                )
                nc.vector.tensor_scalar_mul(
                    out=oout[:, m, :],
                    in0=trp[:, m * tw : m * tw + D],
                    scalar1=rec,
                )
            nc.sync.dma_start(
                out=o_d[b, i * QW : (i + 1) * QW, :].rearrange(
                    "(m p) d -> p m d", p=P
                ),
                in_=oout,
            )

        for b in range(BL):
            qts, kts, vvs = load_batch(b)
            for i in range(NQT):
                compute_qtile(b, i, qts, kts, vvs)

    return nc


_NC_CACHE = None


def _get_nc():
    global _NC_CACHE
    if _NC_CACHE is None:
        nc = build_nc()
        nc.finalize()
        _NC_CACHE = nc
    return _NC_CACHE


def prep_inputs(queries, keys, values):
    """Host-side shard + layout prep (numpy only)."""
    q = np.asarray(queries, dtype=np.float32)
    k = np.asarray(keys, dtype=np.float32)
    v = np.asarray(values, dtype=np.float32)
    assert q.shape == (B, T, D), q.shape
    qT = q.transpose(0, 2, 1)                                  # [B, 64, T]
    q2 = np.concatenate([qT, qT], axis=1)                      # [B, 128, T]
    kT = k.transpose(0, 2, 1).reshape(B, D, NCH, P)            # [B, 64, 16, 128]
    k2 = np.concatenate(
        [
            kT[:, :, 0::2, :].reshape(B, D, T // 2),
            kT[:, :, 1::2, :].reshape(B, D, T // 2),
        ],
        axis=1,
    )                                                          # [B, 128, T/2]
    va = np.concatenate([v, np.ones((B, T, 1), np.float32)], axis=-1)
    if BF16_PV:
        va = va.astype(ml_dtypes.bfloat16)
    if BF16_MM:
        q2 = q2.astype(ml_dtypes.bfloat16)
        k2 = k2.astype(ml_dtypes.bfloat16)
    q2 = np.ascontiguousarray(q2)
    k2 = np.ascontiguousarray(k2)
    va = np.ascontiguousarray(va)
    return [
        {
            "q2": q2[c * BL : (c + 1) * BL],
            "k2": k2[c * BL : (c + 1) * BL],
            "v": va[c * BL : (c + 1) * BL],
        }
        for c in range(NCORES)
    ]


def run(queries, keys, values, trace=False):
    nc = _get_nc()
    core_ids = list(range(NCORES))
    in_maps = prep_inputs(queries, keys, values)
    try:
        res = run_bass_kernel_spmd(nc, in_maps, core_ids, trace=trace)
    except Exception:
        # transient NRT_EXEC_UNIT_UNRECOVERABLE has been observed once in
        # ~30 runs; a straight retry recovers
        res = run_bass_kernel_spmd(nc, in_maps, core_ids, trace=trace)
    out = np.concatenate([res.results[c]["o"] for c in core_ids], axis=0)
    return out.astype(np.float32), res


def kernel(queries, keys, values):
    out, _ = run(queries, keys, values, trace=False)
    return out



# revision 7
# speedup vs baseline: 1.3428x; 1.1349x over previous
"""Causal dot-product attention for Trainium2 (Bass/Tile), 8-core SPMD.

Problem: B=32, T=2048, D=64 fp32.  reference:
    O = softmax(mask(Q K^T / sqrt(D))) V      (causal mask, per batch)

Sharding: pure batch parallelism - 4 batches per NeuronCore, no collectives.

Per-core algorithm (flash-style; no online rescale needed: scores ~ N(0,1),
so exp() is computed directly with a constant stability shift that cancels
in the softmax):

  S^T layout (= K Q^T) so the PV contraction (over key positions) lands on
  the partition dim.  The S^T contraction dim is only D=64, so pairs of key
  chunks are packed into the two 64-row halves of the PE array
  (tile_position row packing, auto-derived from operand base partitions)
  and run concurrently - the concurrent pair MUST target different PSUM
  banks.  Host-side prep supplies Q^T duplicated into both partition halves
  and K^T with even/odd chunks interleaved, plus the ones-augmented V.

  All matmuls run in bf16 (fp32 PSUM accumulation): FWL (fast weight load)
  halves LDWEIGHTS, the dense bf16 stream keeps the HAM clock gate at
  2.4 GHz (fp32r mode measured the PE throttled to 1.2 GHz for ~60% of the
  kernel), and input DMA halves.  Scores keep ~0.3% accuracy through the
  fp32 PSUM + fp32 exp path; overall output rel err ~3e-3 (budget 2e-2).

  Per batch (16 key chunks of 128, 4 query tiles of 512):
    for each q-tile i, key-chunk pair u (off-diagonal lead, then diagonal
    pairs so the DVE mask latency hides under the off-diagonal pipeline):
      S^T pair -> one PSUM [128, 2, 512] tile (half-width N=256 for the
      outer diagonal pair), one ACT exp(s/8 - 2) pass PSUM->SBUF (bf16),
      DVE multiplies by a precomputed 0/1 triangle mask zero the causal
      diagonal blocks (tri blocks only; fully-masked blocks are skipped
      outright), then PV in direct-O form: for each 128-query subchunk s,
      matmul(lhsT=pexp[:, h, s*128:+128], rhs=V_chunk[128, 65]) accumulates
      O[q, d] (+ softmax sums in column 64) straight into PSUM [128, 4, 65]
      - N=65 streaming columns, full 128x128 array use, NO transposes.
    epilogue per q-tile: DVE reciprocal of the 4 sums columns, 4
    tensor_scalar multiplies PSUM->SBUF, DMA out.

A dense bf16 matmul burst on dummy data runs during the initial input-DMA
stall to open the HAM clock gate before the real stream starts.
"""

import os

# Standard recovery knob: reset NeuronCores at runtime init (harmless on a
# healthy device, helps if a previous run left cores wedged). Set before
# backend init; a no-op if the caller already configured it.
os.environ.setdefault("NEURON_RT_RESET_CORES", "1")

import ml_dtypes
import numpy as np

import concourse.bacc as bacc
import concourse.mybir as mybir
import concourse.tile as tile
from concourse.bass_utils import run_bass_kernel_spmd

B, T, D = 32, 2048, 64
NCORES = 8
BL = B // NCORES            # batches per core
P = 128                     # partitions / key-chunk size
NCH = T // P                # key chunks per batch (16)
QW = 512                    # query-tile width
NQT = T // QW               # query tiles per batch (4)
NSUB = QW // P              # 128-query subchunks per q-tile (4)
SCALE = 1.0 / np.sqrt(D)    # 0.125
EBIAS = -2.0                # stability shift inside exp(); cancels in softmax

F32 = mybir.dt.float32
BF16 = mybir.dt.bfloat16

PREWARM = os.environ.get("ATTN_PREWARM", "1") == "1"
PREWARM_N = int(os.environ.get("ATTN_PREWARM_N", "12"))


def build_nc():
    from contextlib import ExitStack

    nc = bacc.Bacc()
    # host-prepped inputs (bf16):
    #   q2: Q^T duplicated into both partition halves      [BL, 128, T]
    #   k2: K^T, even chunks rows 0:64, odd rows 64:128    [BL, 128, T/2]
    #   v:  V with ones column                             [BL, T, D+1]
    q2_d = nc.dram_tensor("q2", [BL, P, T], BF16, kind="ExternalInput")
    k2_d = nc.dram_tensor("k2", [BL, P, T // 2], BF16, kind="ExternalInput")
    v_d = nc.dram_tensor("v", [BL, T, D + 1], BF16, kind="ExternalInput")
    o_d = nc.dram_tensor("o", [BL, T, D], F32, kind="ExternalOutput")

    with tile.TileContext(nc) as tc, ExitStack() as ctx:
        singles = ctx.enter_context(tc.tile_pool(name="singles", bufs=1))
        wpool = ctx.enter_context(tc.tile_pool(name="wts", bufs=4))
        pepool = ctx.enter_context(tc.tile_pool(name="pexp", bufs=8))
        oout_pool = ctx.enter_context(tc.tile_pool(name="oout", bufs=3))
        rec_pool = ctx.enter_context(tc.tile_pool(name="rec", bufs=8))
        st_ps = ctx.enter_context(tc.tile_pool(name="stps", bufs=3, space="PSUM"))
        op_ps = ctx.enter_context(tc.tile_pool(name="opps", bufs=2, space="PSUM"))

        ebias = singles.tile([P, 1], F32)
        nc.vector.memset(ebias, EBIAS)
        # precomputed 0/1 causal triangle mask (keep where f >= p), applied
        # by DVE multiplies to the diagonal 128x128 blocks only
        tri0f = singles.tile([P, P], F32)
        nc.vector.memset(tri0f, 1.0)
        nc.gpsimd.affine_select(
            out=tri0f, in_=tri0f, compare_op=mybir.AluOpType.is_ge, fill=0.0,
            base=0, channel_multiplier=-1, pattern=[[1, P]],
        )
        tri0 = singles.tile([P, P], BF16)
        nc.vector.tensor_copy(out=tri0, in_=tri0f)

        if PREWARM:
            # dense bf16 matmul burst on dummy data, scheduled during the
            # initial input-DMA stall (no data deps): holds the PE busy for
            # >3.4us so the HAM clock gate opens to 2.4 GHz before the real
            # stream starts. Uses an "st" pool slot (released before the
            # third S^T pair needs it) -> no extra PSUM bank.
            wsrc = singles.tile([P, QW], BF16)
            nc.vector.memset(wsrc, 0.5)
            wps = st_ps.tile([P, 2, QW], F32, tag="st", name="warm")
            for _ in range(PREWARM_N):
                nc.tensor.matmul(
                    out=wps[:, 0, :], lhsT=wsrc[:, 0:P], rhs=wsrc,
                    start=True, stop=True,
                )

        def load_batch(b):
            qt = wpool.tile([P, T], BF16, tag="qt", name=f"qt{b}")
            nc.sync.dma_start(out=qt, in_=q2_d[b])
            kt = wpool.tile([P, T // 2], BF16, tag="kt", name=f"kt{b}")
            nc.sync.dma_start(out=kt, in_=k2_d[b])
            vv = wpool.tile([P, NCH, D + 1], BF16, tag="vv", name=f"vv{b}")
            vsrc = v_d[b].rearrange("(c p) d -> p c d", p=P)
            nc.sync.dma_start(out=vv, in_=vsrc)
            return qt, kt, vv

        def compute_qtile(b, i, qt, kt, vv):
            # O accumulator: [q_sub 128, s, d+1] - 4 subchunks in ONE PSUM
            # bank; the single start=True matmul clears the whole bank.
            op = op_ps.tile([P, NSUB, D + 1], F32, tag="op", name=f"op{b}_{i}")
            # off-diagonal lead (shortest chain to the start=True PV), then
            # the diagonal pairs so their mask latency hides under the
            # remaining off-diagonal pipeline
            if i == 0:
                order = [0, 1]
            else:
                order = [0, 2 * i, 2 * i + 1] + list(range(1, 2 * i))
            first = True
            # (pair u, h) -> list of (subchunk s, mask) PV jobs; mask is
            # None (full block), tri0 (diagonal block), or "skip" handled
            # by omission.  chunk index c = 2u + h covers keys
            # [c*128, (c+1)*128); subchunk s covers queries
            # [i*512 + s*128, +128) -> fully masked iff c > 4i + s,
            # diagonal iff c == 4i + s.
            jobs = {}
            for oidx, u in enumerate(order):
                for h in range(2):
                    c = 2 * u + h
                    lst = []
                    for s in range(NSUB):
                        if c > 4 * i + s:
                            continue                      # fully masked
                        lst.append((s, tri0 if c == 4 * i + s else None))
                    jobs[(u, h)] = lst
            last_u = order[-1]
            for oidx, u in enumerate(order):
                half = u == 2 * i + 1                     # outer diagonal pair
                w = 256 if half else QW                   # live q-width
                lo = QW - w                               # first live q col
                stp = st_ps.tile([P, 2, QW], F32, tag="st", name=f"st{b}_{i}_{u}")
                pexp = pepool.tile([P, 2, QW], BF16, tag="pe", name=f"pe{b}_{i}_{u}")
                for h in range(2):
                    # concurrent row-packed matmuls target DIFFERENT PSUM
                    # banks (stp[:, h] is bank h of the tile)
                    nc.tensor.matmul(
                        out=stp[:, h, lo : lo + w],
                        lhsT=kt[h * D : (h + 1) * D, u * P : (u + 1) * P],
                        rhs=qt[h * D : (h + 1) * D, i * QW + lo : (i + 1) * QW],
                        start=True,
                        stop=True,
                    )
                nc.scalar.activation(
                    out=pexp[:, :, lo : lo + w],
                    in_=stp[:, :, lo : lo + w],
                    func=mybir.ActivationFunctionType.Exp,
                    bias=ebias,
                    scale=SCALE,
                )
                # DVE triangle masks on the diagonal 128-blocks
                for h in range(2):
                    for s, msk in jobs[(u, h)]:
                        if msk is not None:
                            blk = pexp[:, h, s * P : (s + 1) * P]
                            nc.vector.tensor_mul(out=blk, in0=blk, in1=msk)
                # PV, direct-O: unmasked subchunks first (depend only on
                # exp), masked ones last (wait for the DVE mask)
                seq = []
                for h in range(2):
                    seq += [(h, s) for s, m in jobs[(u, h)] if m is None]
                for h in range(2):
                    seq += [(h, s) for s, m in jobs[(u, h)] if m is not None]
                for n, (h, s) in enumerate(seq):
                    stop = u == last_u and n == len(seq) - 1
                    nc.tensor.matmul(
                        out=op[:, s, :],
                        lhsT=pexp[:, h, s * P : (s + 1) * P],
                        rhs=vv[:, 2 * u + h, :],
                        start=first,
                        stop=stop,
                    )
                    first = False
            # epilogue: normalize by the sums column, straight from PSUM
            rec = rec_pool.tile([P, NSUB], F32, tag="rec", name=f"rec{b}_{i}")
            nc.vector.reciprocal(out=rec, in_=op[:, :, D])
            oout = oout_pool.tile([P, NSUB, D], F32, tag="oo", name=f"oo{b}_{i}")
            for s in range(NSUB):
                nc.vector.tensor_scalar_mul(
                    out=oout[:, s, :],
                    in0=op[:, s, 0:D],
                    scalar1=rec[:, s : s + 1],
                )
            nc.sync.dma_start(
                out=o_d[b, i * QW : (i + 1) * QW, :].rearrange(
                    "(m p) d -> p m d", p=P
                ),
                in_=oout,
            )

        for b in range(BL):
            qt, kt, vv = load_batch(b)
            for i in range(NQT):
                compute_qtile(b, i, qt, kt, vv)

    return nc


_NC_CACHE = None


def _get_nc():
    global _NC_CACHE
    if _NC_CACHE is None:
        nc = build_nc()
        nc.finalize()
        _NC_CACHE = nc
    return _NC_CACHE


def prep_inputs(queries, keys, values):
    """Host-side shard + layout prep (numpy only)."""
    q = np.asarray(queries, dtype=np.float32)
    k = np.asarray(keys, dtype=np.float32)
    v = np.asarray(values, dtype=np.float32)
    assert q.shape == (B, T, D), q.shape
    qT = q.transpose(0, 2, 1)                                  # [B, 64, T]
    q2 = np.concatenate([qT, qT], axis=1)                      # [B, 128, T]
    kT = k.transpose(0, 2, 1).reshape(B, D, NCH, P)            # [B, 64, 16, 128]
    k2 = np.concatenate(
        [
            kT[:, :, 0::2, :].reshape(B, D, T // 2),
            kT[:, :, 1::2, :].reshape(B, D, T // 2),
        ],
        axis=1,
    )                                                          # [B, 128, T/2]
    va = np.concatenate([v, np.ones((B, T, 1), np.float32)], axis=-1)
    q2 = np.ascontiguousarray(q2.astype(ml_dtypes.bfloat16))
    k2 = np.ascontiguousarray(k2.astype(ml_dtypes.bfloat16))
    va = np.ascontiguousarray(va.astype(ml_dtypes.bfloat16))
    return [
        {
            "q2": q2[c * BL : (c + 1) * BL],
            "k2": k2[c * BL : (c + 1) * BL],
            "v": va[c * BL : (c + 1) * BL],
        }
        for c in range(NCORES)
    ]


def run(queries, keys, values, trace=False):
    nc = _get_nc()
    core_ids = list(range(NCORES))
    in_maps = prep_inputs(queries, keys, values)
    try:
        res = run_bass_kernel_spmd(nc, in_maps, core_ids, trace=trace)
    except Exception:
        # transient NRT_EXEC_UNIT_UNRECOVERABLE has been observed once in
        # ~30 runs; a straight retry recovers
        res = run_bass_kernel_spmd(nc, in_maps, core_ids, trace=trace)
    out = np.concatenate([res.results[c]["o"] for c in core_ids], axis=0)
    return out.astype(np.float32), res


def kernel(queries, keys, values):
    out, _ = run(queries, keys, values, trace=False)
    return out


# revision 10
# speedup vs baseline: 1.4406x; 1.0728x over previous
"""Causal dot-product attention for Trainium2 (Bass/Tile), 8-core SPMD.

Problem: B=32, T=2048, D=64 fp32.  reference:
    O = softmax(mask(Q K^T / sqrt(D))) V      (causal mask, per batch)

Sharding: pure batch parallelism - 4 batches per NeuronCore, no collectives.

Per-core algorithm (flash-style; no online rescale needed: scores ~ N(0,1),
so exp() is computed directly with a constant stability shift that cancels
in the softmax):

  S^T layout (= K Q^T) so the PV contraction (over key positions) lands on
  the partition dim.  The S^T contraction dim is only D=64, so pairs of key
  chunks are packed into the two 64-row halves of the PE array
  (tile_position row packing, auto-derived from operand base partitions)
  and run concurrently - the concurrent pair MUST target different PSUM
  banks.  Host-side prep supplies Q^T duplicated into both partition halves
  and K^T with even/odd chunks interleaved, plus the ones-augmented V.

  All matmuls run in bf16 (fp32 PSUM accumulation): FWL (fast weight load)
  halves LDWEIGHTS, the dense bf16 stream keeps the HAM clock gate at
  2.4 GHz (fp32r mode measured the PE throttled to 1.2 GHz for ~60% of the
  kernel), and input DMA halves.  Scores keep ~0.3% accuracy through the
  fp32 PSUM + fp32 exp path; overall output rel err ~3e-3 (budget 2e-2).

  Per batch (16 key chunks of 128, 4 query tiles of 512):
    for each q-tile i, key-chunk pair u (off-diagonal lead, then diagonal
    pairs so the DVE mask latency hides under the off-diagonal pipeline):
      S^T pair -> one PSUM [128, 2, 512] tile (half-width N=256 for the
      outer diagonal pair), one ACT exp(s/8 - 2) pass PSUM->SBUF (bf16),
      DVE multiplies by a precomputed 0/1 triangle mask zero the causal
      diagonal blocks (tri blocks only; fully-masked blocks are skipped
      outright), then PV in direct-O form: for each 128-query subchunk s,
      matmul(lhsT=pexp[:, h, s*128:+128], rhs=V_chunk[128, 65]) accumulates
      O[q, d] (+ softmax sums in column 64) straight into PSUM [128, 4, 65]
      - N=65 streaming columns, full 128x128 array use, NO transposes.
    epilogue per q-tile: DVE reciprocal of the 4 sums columns, 4
    tensor_scalar multiplies PSUM->SBUF, DMA out.

A dense bf16 matmul burst on dummy data runs during the initial input-DMA
stall to open the HAM clock gate before the real stream starts.
"""

import os

# Standard recovery knob: reset NeuronCores at runtime init (harmless on a
# healthy device, helps if a previous run left cores wedged). Set before
# backend init; a no-op if the caller already configured it.
os.environ.setdefault("NEURON_RT_RESET_CORES", "1")

import ml_dtypes
import numpy as np

import concourse.bacc as bacc
import concourse.mybir as mybir
import concourse.tile as tile
from concourse.bass_utils import run_bass_kernel_spmd

B, T, D = 32, 2048, 64
NCORES = 8
BL = B // NCORES            # batches per core
P = 128                     # partitions / key-chunk size
NCH = T // P                # key chunks per batch (16)
QW = 512                    # query-tile width
NQT = T // QW               # query tiles per batch (4)
NSUB = QW // P              # 128-query subchunks per q-tile (4)
SCALE = 1.0 / np.sqrt(D)    # 0.125
EBIAS = -2.0                # stability shift inside exp(); cancels in softmax

F32 = mybir.dt.float32
BF16 = mybir.dt.bfloat16

PREWARM = os.environ.get("ATTN_PREWARM", "1") == "1"
PREWARM_N = int(os.environ.get("ATTN_PREWARM_N", "7"))
# Offload the exp of alternating off-diagonal pairs to the DVE via the
# Schraudolph bit trick targeting bf16: bf16bits(exp(s/8-2)) ~
# int16(s*SCH_A + SCH_B).  Max per-weight rel err ~3%; softmax
# normalization cancels most of it (measured ~2.5e-3 end-to-end with half
# the keys on this path).  Balances the two elementwise engines: ACT keeps
# ~60% of the exp columns, DVE takes ~40%.
DVE_EXP = os.environ.get("ATTN_DVE_EXP", "1") == "1"
SCH_A = 0.125 * (2.0**7) / np.log(2.0)            # 23.0831...
SCH_B = 127.0 * 128.0 - 2.0 * (2.0**7) / np.log(2.0) - 5.5
# GPSIMD (Pool) applies the causal triangle masks; it is otherwise idle and
# this frees the DVE for the exp offload.
POOL_MASK = os.environ.get("ATTN_POOL_MASK", "1") == "1"
I16 = mybir.dt.int16


def build_nc():
    from contextlib import ExitStack

    nc = bacc.Bacc()
    # host-prepped inputs (bf16):
    #   q2: Q^T duplicated into both partition halves      [BL, 128, T]
    #   k2: K^T, even chunks rows 0:64, odd rows 64:128    [BL, 128, T/2]
    #   v:  V with ones column                             [BL, T, D+1]
    q2_d = nc.dram_tensor("q2", [BL, P, T], BF16, kind="ExternalInput")
    k2_d = nc.dram_tensor("k2", [BL, P, T // 2], BF16, kind="ExternalInput")
    v_d = nc.dram_tensor("v", [BL, T, D + 1], BF16, kind="ExternalInput")
    o_d = nc.dram_tensor("o", [BL, T, D], F32, kind="ExternalOutput")

    with tile.TileContext(nc) as tc, ExitStack() as ctx:
        singles = ctx.enter_context(tc.tile_pool(name="singles", bufs=1))
        wpool = ctx.enter_context(tc.tile_pool(name="wts", bufs=4))
        pepool = ctx.enter_context(tc.tile_pool(name="pexp", bufs=8))
        oout_pool = ctx.enter_context(tc.tile_pool(name="oout", bufs=3))
        rec_pool = ctx.enter_context(tc.tile_pool(name="rec", bufs=8))
        st_ps = ctx.enter_context(tc.tile_pool(name="stps", bufs=3, space="PSUM"))
        op_ps = ctx.enter_context(tc.tile_pool(name="opps", bufs=2, space="PSUM"))

        ebias = singles.tile([P, 1], F32)
        nc.vector.memset(ebias, EBIAS)
        # precomputed 0/1 causal triangle mask (keep where f >= p), applied
        # by DVE multiplies to the diagonal 128x128 blocks only
        tri0f = singles.tile([P, P], F32)
        nc.vector.memset(tri0f, 1.0)
        nc.gpsimd.affine_select(
            out=tri0f, in_=tri0f, compare_op=mybir.AluOpType.is_ge, fill=0.0,
            base=0, channel_multiplier=-1, pattern=[[1, P]],
        )
        tri0 = singles.tile([P, P], BF16)
        nc.vector.tensor_copy(out=tri0, in_=tri0f)

        if PREWARM:
            # dense bf16 matmul burst on dummy data, scheduled during the
            # initial input-DMA stall (no data deps): holds the PE busy for
            # >3.4us so the HAM clock gate opens to 2.4 GHz before the real
            # stream starts. Uses an "st" pool slot (released before the
            # third S^T pair needs it) -> no extra PSUM bank.
            wsrc = singles.tile([P, QW], BF16)
            nc.vector.memset(wsrc, 0.5)
            wps = st_ps.tile([P, 2, QW], F32, tag="st", name="warm")
            for _ in range(PREWARM_N):
                nc.tensor.matmul(
                    out=wps[:, 0, :], lhsT=wsrc[:, 0:P], rhs=wsrc,
                    start=True, stop=True,
                )

        def load_batch(b):
            # split the q/k loads so q-tile 0 (pairs 0-1) can start as soon
            # as the small head slices land, instead of waiting for the
            # full-width transfers
            qt = wpool.tile([P, T], BF16, tag="qt", name=f"qt{b}")
            nc.sync.dma_start(out=qt[:, 0:QW], in_=q2_d[b, :, 0:QW])
            kt = wpool.tile([P, T // 2], BF16, tag="kt", name=f"kt{b}")
            nc.sync.dma_start(out=kt[:, 0 : 2 * P], in_=k2_d[b, :, 0 : 2 * P])
            vv = wpool.tile([P, NCH, D + 1], BF16, tag="vv", name=f"vv{b}")
            vsrc = v_d[b].rearrange("(c p) d -> p c d", p=P)
            nc.sync.dma_start(out=vv, in_=vsrc)
            nc.sync.dma_start(out=qt[:, QW:T], in_=q2_d[b, :, QW:T])
            nc.sync.dma_start(out=kt[:, 2 * P :], in_=k2_d[b, :, 2 * P :])
            return qt, kt, vv

        def compute_qtile(b, i, qt, kt, vv):
            # O accumulator: [q_sub 128, s, d+1] - 4 subchunks in ONE PSUM
            # bank; the single start=True matmul clears the whole bank.
            op = op_ps.tile([P, NSUB, D + 1], F32, tag="op", name=f"op{b}_{i}")
            # off-diagonal lead (shortest chain to the start=True PV), then
            # the diagonal pairs so their mask latency hides under the
            # remaining off-diagonal pipeline
            if i == 0:
                order = [0, 1]
            else:
                order = [0, 2 * i, 2 * i + 1] + list(range(1, 2 * i))
            first = True
            # (pair u, h) -> list of (subchunk s, mask) PV jobs; mask is
            # None (full block), tri0 (diagonal block), or "skip" handled
            # by omission.  chunk index c = 2u + h covers keys
            # [c*128, (c+1)*128); subchunk s covers queries
            # [i*512 + s*128, +128) -> fully masked iff c > 4i + s,
            # diagonal iff c == 4i + s.
            jobs = {}
            for oidx, u in enumerate(order):
                for h in range(2):
                    c = 2 * u + h
                    lst = []
                    for s in range(NSUB):
                        if c > 4 * i + s:
                            continue                      # fully masked
                        lst.append((s, tri0 if c == 4 * i + s else None))
                    jobs[(u, h)] = lst
            last_u = order[-1]
            for oidx, u in enumerate(order):
                half = u == 2 * i + 1                     # outer diagonal pair
                w = 256 if half else QW                   # live q-width
                lo = QW - w                               # first live q col
                stp = st_ps.tile([P, 2, QW], F32, tag="st", name=f"st{b}_{i}_{u}")
                pexp = pepool.tile([P, 2, QW], BF16, tag="pe", name=f"pe{b}_{i}_{u}")
                for h in range(2):
                    # concurrent row-packed matmuls target DIFFERENT PSUM
                    # banks (stp[:, h] is bank h of the tile)
                    nc.tensor.matmul(
                        out=stp[:, h, lo : lo + w],
                        lhsT=kt[h * D : (h + 1) * D, u * P : (u + 1) * P],
                        rhs=qt[h * D : (h + 1) * D, i * QW + lo : (i + 1) * QW],
                        start=True,
                        stop=True,
                    )
                if DVE_EXP and u < 2 * i and u % 2 == 1:
                    # Schraudolph bf16 exp on the DVE: one fused
                    # multiply-add straight into the bf16 bit pattern
                    nc.vector.tensor_scalar(
                        out=pexp[:, :, lo : lo + w].bitcast(I16),
                        in0=stp[:, :, lo : lo + w],
                        scalar1=SCH_A,
                        scalar2=SCH_B,
                        op0=mybir.AluOpType.mult,
                        op1=mybir.AluOpType.add,
                    )
                else:
                    nc.scalar.activation(
                        out=pexp[:, :, lo : lo + w],
                        in_=stp[:, :, lo : lo + w],
                        func=mybir.ActivationFunctionType.Exp,
                        bias=ebias,
                        scale=SCALE,
                    )
                # triangle masks on the diagonal 128-blocks (GPSIMD; it is
                # otherwise idle)
                meng = nc.gpsimd if POOL_MASK else nc.vector
                for h in range(2):
                    for s, msk in jobs[(u, h)]:
                        if msk is not None:
                            blk = pexp[:, h, s * P : (s + 1) * P]
                            meng.tensor_mul(out=blk, in0=blk, in1=msk)
                # PV, direct-O: unmasked subchunks first (depend only on
                # exp), masked ones last (wait for the DVE mask)
                seq = []
                for h in range(2):
                    seq += [(h, s) for s, m in jobs[(u, h)] if m is None]
                for h in range(2):
                    seq += [(h, s) for s, m in jobs[(u, h)] if m is not None]
                for n, (h, s) in enumerate(seq):
                    stop = u == last_u and n == len(seq) - 1
                    nc.tensor.matmul(
                        out=op[:, s, :],
                        lhsT=pexp[:, h, s * P : (s + 1) * P],
                        rhs=vv[:, 2 * u + h, :],
                        start=first,
                        stop=stop,
                    )
                    first = False
            # epilogue: normalize by the sums column, straight from PSUM
            rec = rec_pool.tile([P, NSUB], F32, tag="rec", name=f"rec{b}_{i}")
            nc.vector.reciprocal(out=rec, in_=op[:, :, D])
            oout = oout_pool.tile([P, NSUB, D], F32, tag="oo", name=f"oo{b}_{i}")
            for s in range(NSUB):
                nc.vector.tensor_scalar_mul(
                    out=oout[:, s, :],
                    in0=op[:, s, 0:D],
                    scalar1=rec[:, s : s + 1],
                )
            nc.sync.dma_start(
                out=o_d[b, i * QW : (i + 1) * QW, :].rearrange(
                    "(m p) d -> p m d", p=P
                ),
                in_=oout,
            )

        for b in range(BL):
            qt, kt, vv = load_batch(b)
            for i in range(NQT):
                compute_qtile(b, i, qt, kt, vv)

    return nc


_NC_CACHE = None


def _get_nc():
    global _NC_CACHE
    if _NC_CACHE is None:
        nc = build_nc()
        nc.finalize()
        _NC_CACHE = nc
    return _NC_CACHE


def prep_inputs(queries, keys, values):
    """Host-side shard + layout prep (numpy only)."""
    q = np.asarray(queries, dtype=np.float32)
    k = np.asarray(keys, dtype=np.float32)
    v = np.asarray(values, dtype=np.float32)
    assert q.shape == (B, T, D), q.shape
    qT = q.transpose(0, 2, 1)                                  # [B, 64, T]
    q2 = np.concatenate([qT, qT], axis=1)                      # [B, 128, T]
    kT = k.transpose(0, 2, 1).reshape(B, D, NCH, P)            # [B, 64, 16, 128]
    k2 = np.concatenate(
        [
            kT[:, :, 0::2, :].reshape(B, D, T // 2),
            kT[:, :, 1::2, :].reshape(B, D, T // 2),
        ],
        axis=1,
    )                                                          # [B, 128, T/2]
    va = np.concatenate([v, np.ones((B, T, 1), np.float32)], axis=-1)
    q2 = np.ascontiguousarray(q2.astype(ml_dtypes.bfloat16))
    k2 = np.ascontiguousarray(k2.astype(ml_dtypes.bfloat16))
    va = np.ascontiguousarray(va.astype(ml_dtypes.bfloat16))
    return [
        {
            "q2": q2[c * BL : (c + 1) * BL],
            "k2": k2[c * BL : (c + 1) * BL],
            "v": va[c * BL : (c + 1) * BL],
        }
        for c in range(NCORES)
    ]


def run(queries, keys, values, trace=False):
    nc = _get_nc()
    core_ids = list(range(NCORES))
    in_maps = prep_inputs(queries, keys, values)
    try:
        res = run_bass_kernel_spmd(nc, in_maps, core_ids, trace=trace)
    except Exception:
        # transient NRT_EXEC_UNIT_UNRECOVERABLE has been observed once in
        # ~30 runs; a straight retry recovers
        res = run_bass_kernel_spmd(nc, in_maps, core_ids, trace=trace)
    out = np.concatenate([res.results[c]["o"] for c in core_ids], axis=0)
    return out.astype(np.float32), res


def kernel(queries, keys, values):
    out, _ = run(queries, keys, values, trace=False)
    return out


# revision 14
# speedup vs baseline: 1.4625x; 1.0152x over previous
"""Causal dot-product attention for Trainium2 (Bass/Tile), 8-core SPMD.

Problem: B=32, T=2048, D=64 fp32.  reference:
    O = softmax(mask(Q K^T / sqrt(D))) V      (causal mask, per batch)

Sharding: pure batch parallelism - 4 batches per NeuronCore, no collectives.

Per-core algorithm (flash-style; no online rescale needed: scores ~ N(0,1),
so exp() is computed directly with a constant stability shift that cancels
in the softmax):

  S^T layout (= K Q^T) so the PV contraction (over key positions) lands on
  the partition dim.  The S^T contraction dim is only D=64, so pairs of key
  chunks are packed into the two 64-row halves of the PE array
  (tile_position row packing, auto-derived from operand base partitions)
  and run concurrently - the concurrent pair MUST target different PSUM
  banks.  Host-side prep supplies Q^T duplicated into both partition halves
  and K^T with even/odd chunks interleaved, plus the ones-augmented V.

  All matmuls run in bf16 (fp32 PSUM accumulation): FWL (fast weight load)
  halves LDWEIGHTS, the dense bf16 stream keeps the HAM clock gate at
  2.4 GHz (fp32r mode measured the PE throttled to 1.2 GHz for ~60% of the
  kernel), and input DMA halves.  Scores keep ~0.3% accuracy through the
  fp32 PSUM + fp32 exp path; overall output rel err ~3e-3 (budget 2e-2).

  Per batch (16 key chunks of 128, 4 query tiles of 512):
    for each q-tile i, key-chunk pair u (off-diagonal lead, then diagonal
    pairs so the DVE mask latency hides under the off-diagonal pipeline):
      S^T pair -> one PSUM [128, 2, 512] tile (half-width N=256 for the
      outer diagonal pair), one ACT exp(s/8 - 2) pass PSUM->SBUF (bf16),
      DVE multiplies by a precomputed 0/1 triangle mask zero the causal
      diagonal blocks (tri blocks only; fully-masked blocks are skipped
      outright), then PV in direct-O form: for each 128-query subchunk s,
      matmul(lhsT=pexp[:, h, s*128:+128], rhs=V_chunk[128, 65]) accumulates
      O[q, d] (+ softmax sums in column 64) straight into PSUM [128, 4, 65]
      - N=65 streaming columns, full 128x128 array use, NO transposes.
    epilogue per q-tile: DVE reciprocal of the 4 sums columns, 4
    tensor_scalar multiplies PSUM->SBUF, DMA out.

A dense bf16 matmul burst on dummy data runs during the initial input-DMA
stall to open the HAM clock gate before the real stream starts.
"""

import os

# Standard recovery knob: reset NeuronCores at runtime init (harmless on a
# healthy device, helps if a previous run left cores wedged). Set before
# backend init; a no-op if the caller already configured it.
os.environ.setdefault("NEURON_RT_RESET_CORES", "1")

import ml_dtypes
import numpy as np

import concourse.bacc as bacc
import concourse.mybir as mybir
import concourse.tile as tile
from concourse.bass_utils import run_bass_kernel_spmd

B, T, D = 32, 2048, 64
NCORES = 8
BL = B // NCORES            # batches per core
P = 128                     # partitions / key-chunk size
NCH = T // P                # key chunks per batch (16)
QW = 512                    # query-tile width
NQT = T // QW               # query tiles per batch (4)
NSUB = QW // P              # 128-query subchunks per q-tile (4)
SCALE = 1.0 / np.sqrt(D)    # 0.125
EBIAS = -2.0                # stability shift inside exp(); cancels in softmax

F32 = mybir.dt.float32
BF16 = mybir.dt.bfloat16

PREWARM = os.environ.get("ATTN_PREWARM", "1") == "1"
PREWARM_N = int(os.environ.get("ATTN_PREWARM_N", "7"))
# Offload the exp of alternating off-diagonal pairs to the DVE via the
# Schraudolph bit trick targeting bf16: bf16bits(exp(s/8-2)) ~
# int16(s*SCH_A + SCH_B).  Max per-weight rel err ~3%; softmax
# normalization cancels most of it (measured ~2.5e-3 end-to-end with half
# the keys on this path).  Balances the two elementwise engines: ACT keeps
# ~60% of the exp columns, DVE takes ~40%.
DVE_EXP = os.environ.get("ATTN_DVE_EXP", "1") == "1"
SCH_A = 0.125 * (2.0**7) / np.log(2.0)            # 23.0831...
SCH_B = 127.0 * 128.0 - 2.0 * (2.0**7) / np.log(2.0) - 5.5
# GPSIMD (Pool) applies the causal triangle masks; it is otherwise idle and
# this frees the DVE for the exp offload.
POOL_MASK = os.environ.get("ATTN_POOL_MASK", "1") == "1"
# HAM keeper: one dummy bf16 matmul per pair into the stp bank the next
# S^T is about to clear (start=True wipes it).  Raises the PE streaming
# duty cycle above the HAM activity threshold so the clock stays at
# 2.4 GHz instead of oscillating with 1.2 GHz windows.
HEAT_W = int(os.environ.get("ATTN_HEAT_W", "512"))
HEAT_EVERY = int(os.environ.get("ATTN_HEAT_EVERY", "1"))
I16 = mybir.dt.int16


def build_nc():
    from contextlib import ExitStack

    nc = bacc.Bacc()
    # host-prepped inputs (bf16):
    #   q2: Q^T duplicated into both partition halves      [BL, 128, T]
    #   k2: K^T, even chunks rows 0:64, odd rows 64:128    [BL, 128, T/2]
    #   v:  V with ones column                             [BL, T, D+1]
    q2_d = nc.dram_tensor("q2", [BL, P, T], BF16, kind="ExternalInput")
    k2_d = nc.dram_tensor("k2", [BL, P, T // 2], BF16, kind="ExternalInput")
    v_d = nc.dram_tensor("v", [BL, T, D + 1], BF16, kind="ExternalInput")
    o_d = nc.dram_tensor("o", [BL, T, D], F32, kind="ExternalOutput")

    with tile.TileContext(nc) as tc, ExitStack() as ctx:
        singles = ctx.enter_context(tc.tile_pool(name="singles", bufs=1))
        wpool = ctx.enter_context(tc.tile_pool(name="wts", bufs=4))
        pepool = ctx.enter_context(tc.tile_pool(name="pexp", bufs=8))
        oout_pool = ctx.enter_context(tc.tile_pool(name="oout", bufs=3))
        rec_pool = ctx.enter_context(tc.tile_pool(name="rec", bufs=8))
        st_ps = ctx.enter_context(tc.tile_pool(name="stps", bufs=3, space="PSUM"))
        op_ps = ctx.enter_context(tc.tile_pool(name="opps", bufs=2, space="PSUM"))

        ebias = singles.tile([P, 1], F32)
        nc.vector.memset(ebias, EBIAS)
        # precomputed 0/1 causal triangle mask (keep where f >= p), applied
        # by DVE multiplies to the diagonal 128x128 blocks only
        tri0f = singles.tile([P, P], F32)
        nc.vector.memset(tri0f, 1.0)
        nc.gpsimd.affine_select(
            out=tri0f, in_=tri0f, compare_op=mybir.AluOpType.is_ge, fill=0.0,
            base=0, channel_multiplier=-1, pattern=[[1, P]],
        )
        tri0 = singles.tile([P, P], BF16)
        nc.vector.tensor_copy(out=tri0, in_=tri0f)

        wsrc = singles.tile([P, QW], BF16)
        nc.vector.memset(wsrc, 0.5)
        if PREWARM:
            # dense bf16 matmul burst on dummy data, scheduled during the
            # initial input-DMA stall (no data deps): holds the PE busy for
            # >3.4us so the HAM clock gate opens to 2.4 GHz before the real
            # stream starts. Uses an "st" pool slot (released before the
            # third S^T pair needs it) -> no extra PSUM bank.
            wps = st_ps.tile([P, 2, QW], F32, tag="st", name="warm")
            for _ in range(PREWARM_N):
                nc.tensor.matmul(
                    out=wps[:, 0, :], lhsT=wsrc[:, 0:P], rhs=wsrc,
                    start=True, stop=True,
                )

        def load_batch(b):
            # split the q/k loads so q-tile 0 (pairs 0-1) can start as soon
            # as the small head slices land, instead of waiting for the
            # full-width transfers
            qt = wpool.tile([P, T], BF16, tag="qt", name=f"qt{b}")
            nc.sync.dma_start(out=qt[:, 0:QW], in_=q2_d[b, :, 0:QW])
            kt = wpool.tile([P, T // 2], BF16, tag="kt", name=f"kt{b}")
            nc.sync.dma_start(out=kt[:, 0 : 2 * P], in_=k2_d[b, :, 0 : 2 * P])
            vv = wpool.tile([P, NCH, D + 1], BF16, tag="vv", name=f"vv{b}")
            vsrc = v_d[b].rearrange("(c p) d -> p c d", p=P)
            nc.sync.dma_start(out=vv, in_=vsrc)
            nc.sync.dma_start(out=qt[:, QW:T], in_=q2_d[b, :, QW:T])
            nc.sync.dma_start(out=kt[:, 2 * P :], in_=k2_d[b, :, 2 * P :])
            return qt, kt, vv

        def compute_qtile(b, i, qt, kt, vv):
            # O accumulator: [q_sub 128, s, d+1] - 4 subchunks in ONE PSUM
            # bank; the single start=True matmul clears the whole bank.
            op = op_ps.tile([P, NSUB, D + 1], F32, tag="op", name=f"op{b}_{i}")
            # off-diagonal lead (shortest chain to the start=True PV), then
            # the diagonal pairs so their mask latency hides under the
            # remaining off-diagonal pipeline
            if i == 0:
                order = [0, 1]
            else:
                order = [0, 2 * i, 2 * i + 1] + list(range(1, 2 * i))
            first = True
            # (pair u, h) -> list of (subchunk s, mask) PV jobs; mask is
            # None (full block), tri0 (diagonal block), or "skip" handled
            # by omission.  chunk index c = 2u + h covers keys
            # [c*128, (c+1)*128); subchunk s covers queries
            # [i*512 + s*128, +128) -> fully masked iff c > 4i + s,
            # diagonal iff c == 4i + s.
            jobs = {}
            for oidx, u in enumerate(order):
                for h in range(2):
                    c = 2 * u + h
                    lst = []
                    for s in range(NSUB):
                        if c > 4 * i + s:
                            continue                      # fully masked
                        lst.append((s, tri0 if c == 4 * i + s else None))
                    jobs[(u, h)] = lst
            last_u = order[-1]
            for oidx, u in enumerate(order):
                half = u == 2 * i + 1                     # outer diagonal pair
                w = 256 if half else QW                   # live q-width
                lo = QW - w                               # first live q col
                stp = st_ps.tile([P, 2, QW], F32, tag="st", name=f"st{b}_{i}_{u}")
                pexp = pepool.tile([P, 2, QW], BF16, tag="pe", name=f"pe{b}_{i}_{u}")
                if HEAT_W and oidx % HEAT_EVERY == 0:
                    # HAM keeper: garbage matmul into the bank the next S^T
                    # clears with start=True; fills the PE idle while it
                    # waits for the elementwise engines
                    nc.tensor.matmul(
                        out=stp[:, 0, 0:HEAT_W], lhsT=wsrc[:, 0:P],
                        rhs=wsrc[:, 0:HEAT_W], start=True, stop=True,
                        skip_group_check=True,
                    )
                for h in range(2):
                    # concurrent row-packed matmuls target DIFFERENT PSUM
                    # banks (stp[:, h] is bank h of the tile)
                    nc.tensor.matmul(
                        out=stp[:, h, lo : lo + w],
                        lhsT=kt[h * D : (h + 1) * D, u * P : (u + 1) * P],
                        rhs=qt[h * D : (h + 1) * D, i * QW + lo : (i + 1) * QW],
                        start=True,
                        stop=True,
                    )
                if DVE_EXP and u < 2 * i and u % 2 == 1:
                    # Schraudolph bf16 exp on the DVE: one fused
                    # multiply-add straight into the bf16 bit pattern
                    nc.vector.tensor_scalar(
                        out=pexp[:, :, lo : lo + w].bitcast(I16),
                        in0=stp[:, :, lo : lo + w],
                        scalar1=SCH_A,
                        scalar2=SCH_B,
                        op0=mybir.AluOpType.mult,
                        op1=mybir.AluOpType.add,
                    )
                else:
                    nc.scalar.activation(
                        out=pexp[:, :, lo : lo + w],
                        in_=stp[:, :, lo : lo + w],
                        func=mybir.ActivationFunctionType.Exp,
                        bias=ebias,
                        scale=SCALE,
                    )
                # triangle masks on the diagonal 128-blocks (GPSIMD; it is
                # otherwise idle)
                meng = nc.gpsimd if POOL_MASK else nc.vector
                for h in range(2):
                    for s, msk in jobs[(u, h)]:
                        if msk is not None:
                            blk = pexp[:, h, s * P : (s + 1) * P]
                            meng.tensor_mul(out=blk, in0=blk, in1=msk)
                # PV, direct-O: unmasked subchunks first (depend only on
                # exp), masked ones last (wait for the DVE mask)
                seq = []
                for h in range(2):
                    seq += [(h, s) for s, m in jobs[(u, h)] if m is None]
                for h in range(2):
                    seq += [(h, s) for s, m in jobs[(u, h)] if m is not None]
                for n, (h, s) in enumerate(seq):
                    stop = u == last_u and n == len(seq) - 1
                    nc.tensor.matmul(
                        out=op[:, s, :],
                        lhsT=pexp[:, h, s * P : (s + 1) * P],
                        rhs=vv[:, 2 * u + h, :],
                        start=first,
                        stop=stop,
                    )
                    first = False
            # epilogue: normalize by the sums column, straight from PSUM;
            # one reciprocal + one broadcast multiply for all 4 subchunks
            rec = rec_pool.tile([P, NSUB], F32, tag="rec", name=f"rec{b}_{i}")
            nc.vector.reciprocal(out=rec, in_=op[:, :, D])
            oout = oout_pool.tile([P, NSUB, D], F32, tag="oo", name=f"oo{b}_{i}")
            nc.vector.tensor_mul(
                out=oout,
                in0=op[:, :, 0:D],
                in1=rec.unsqueeze(2).broadcast_to([P, NSUB, D]),
            )
            nc.sync.dma_start(
                out=o_d[b, i * QW : (i + 1) * QW, :].rearrange(
                    "(m p) d -> p m d", p=P
                ),
                in_=oout,
            )

        for b in range(BL):
            qt, kt, vv = load_batch(b)
            for i in range(NQT):
                compute_qtile(b, i, qt, kt, vv)

    return nc


_NC_CACHE = None


def _get_nc():
    global _NC_CACHE
    if _NC_CACHE is None:
        nc = build_nc()
        nc.finalize()
        _NC_CACHE = nc
    return _NC_CACHE


def prep_inputs(queries, keys, values):
    """Host-side shard + layout prep (numpy only)."""
    q = np.asarray(queries, dtype=np.float32)
    k = np.asarray(keys, dtype=np.float32)
    v = np.asarray(values, dtype=np.float32)
    assert q.shape == (B, T, D), q.shape
    qT = q.transpose(0, 2, 1)                                  # [B, 64, T]
    q2 = np.concatenate([qT, qT], axis=1)                      # [B, 128, T]
    kT = k.transpose(0, 2, 1).reshape(B, D, NCH, P)            # [B, 64, 16, 128]
    k2 = np.concatenate(
        [
            kT[:, :, 0::2, :].reshape(B, D, T // 2),
            kT[:, :, 1::2, :].reshape(B, D, T // 2),
        ],
        axis=1,
    )                                                          # [B, 128, T/2]
    va = np.concatenate([v, np.ones((B, T, 1), np.float32)], axis=-1)
    q2 = np.ascontiguousarray(q2.astype(ml_dtypes.bfloat16))
    k2 = np.ascontiguousarray(k2.astype(ml_dtypes.bfloat16))
    va = np.ascontiguousarray(va.astype(ml_dtypes.bfloat16))
    return [
        {
            "q2": q2[c * BL : (c + 1) * BL],
            "k2": k2[c * BL : (c + 1) * BL],
            "v": va[c * BL : (c + 1) * BL],
        }
        for c in range(NCORES)
    ]


def run(queries, keys, values, trace=False):
    nc = _get_nc()
    core_ids = list(range(NCORES))
    in_maps = prep_inputs(queries, keys, values)
    try:
        res = run_bass_kernel_spmd(nc, in_maps, core_ids, trace=trace)
    except Exception:
        # transient NRT_EXEC_UNIT_UNRECOVERABLE has been observed once in
        # ~30 runs; a straight retry recovers
        res = run_bass_kernel_spmd(nc, in_maps, core_ids, trace=trace)
    out = np.concatenate([res.results[c]["o"] for c in core_ids], axis=0)
    return out.astype(np.float32), res


def kernel(queries, keys, values):
    out, _ = run(queries, keys, values, trace=False)
    return out


# revision 21
# speedup vs baseline: 1.4785x; 1.0109x over previous
"""Causal dot-product attention for Trainium2 (Bass/Tile), 8-core SPMD.

Problem: B=32, T=2048, D=64 fp32.  reference:
    O = softmax(mask(Q K^T / sqrt(D))) V      (causal mask, per batch)

Sharding: pure batch parallelism - 4 batches per NeuronCore, no collectives.

Per-core algorithm (flash-style; no online rescale needed: scores ~ N(0,1),
so exp() is computed directly with a constant stability shift that cancels
in the softmax):

  S^T layout (= K Q^T) so the PV contraction (over key positions) lands on
  the partition dim.  The S^T contraction dim is only D=64, so pairs of key
  chunks are packed into the two 64-row halves of the PE array
  (tile_position row packing, auto-derived from operand base partitions)
  and run concurrently - the concurrent pair MUST target different PSUM
  banks.  Host-side prep supplies Q^T duplicated into both partition halves
  and K^T with even/odd chunks interleaved, plus the ones-augmented V.

  All matmuls run in bf16 (fp32 PSUM accumulation): FWL (fast weight load)
  halves LDWEIGHTS, the dense bf16 stream keeps the HAM clock gate at
  2.4 GHz (fp32r mode measured the PE throttled to 1.2 GHz for ~60% of the
  kernel), and input DMA halves.  Scores keep ~0.3% accuracy through the
  fp32 PSUM + fp32 exp path; overall output rel err ~3e-3 (budget 2e-2).

  Per batch (16 key chunks of 128, 4 query tiles of 512):
    for each q-tile i, key-chunk pair u (off-diagonal lead, then diagonal
    pairs so the DVE mask latency hides under the off-diagonal pipeline):
      S^T pair -> one PSUM [128, 2, 512] tile (half-width N=256 for the
      outer diagonal pair), one ACT exp(s/8 - 2) pass PSUM->SBUF (bf16),
      DVE multiplies by a precomputed 0/1 triangle mask zero the causal
      diagonal blocks (tri blocks only; fully-masked blocks are skipped
      outright), then PV in direct-O form: for each 128-query subchunk s,
      matmul(lhsT=pexp[:, h, s*128:+128], rhs=V_chunk[128, 65]) accumulates
      O[q, d] (+ softmax sums in column 64) straight into PSUM [128, 4, 65]
      - N=65 streaming columns, full 128x128 array use, NO transposes.
    epilogue per q-tile: DVE reciprocal of the 4 sums columns, 4
    tensor_scalar multiplies PSUM->SBUF, DMA out.

A dense bf16 matmul burst on dummy data runs during the initial input-DMA
stall to open the HAM clock gate before the real stream starts.
"""

import os

# Standard recovery knob: reset NeuronCores at runtime init (harmless on a
# healthy device, helps if a previous run left cores wedged). Set before
# backend init; a no-op if the caller already configured it.
os.environ.setdefault("NEURON_RT_RESET_CORES", "1")

import ml_dtypes
import numpy as np

import concourse.bacc as bacc
import concourse.mybir as mybir
import concourse.tile as tile
from concourse.bass_utils import run_bass_kernel_spmd

B, T, D = 32, 2048, 64
NCORES = 8
BL = B // NCORES            # batches per core
P = 128                     # partitions / key-chunk size
NCH = T // P                # key chunks per batch (16)
QW = 512                    # query-tile width
NQT = T // QW               # query tiles per batch (4)
NSUB = QW // P              # 128-query subchunks per q-tile (4)
SCALE = 1.0 / np.sqrt(D)    # 0.125
EBIAS = -2.0                # stability shift inside exp(); cancels in softmax

F32 = mybir.dt.float32
BF16 = mybir.dt.bfloat16

PREWARM = os.environ.get("ATTN_PREWARM", "1") == "1"
PREWARM_N = int(os.environ.get("ATTN_PREWARM_N", "7"))
# Offload the exp of alternating off-diagonal pairs to the DVE via the
# Schraudolph bit trick targeting bf16: bf16bits(exp(s/8-2)) ~
# int16(s*SCH_A + SCH_B).  Max per-weight rel err ~3%; softmax
# normalization cancels most of it (measured ~2.5e-3 end-to-end with half
# the keys on this path).  Balances the two elementwise engines: ACT keeps
# ~60% of the exp columns, DVE takes ~40%.
DVE_EXP = os.environ.get("ATTN_DVE_EXP", "1") == "1"
SCH_A = 0.125 * (2.0**7) / np.log(2.0)            # 23.0831...
SCH_B = 127.0 * 128.0 - 2.0 * (2.0**7) / np.log(2.0) - 5.5
# GPSIMD (Pool) applies the causal triangle masks; it is otherwise idle and
# this frees the DVE for the exp offload.
POOL_MASK = os.environ.get("ATTN_POOL_MASK", "1") == "1"
# HAM keeper: dummy bf16 matmuls per pair into spare rows of the op
# accumulator bank (never read; each q-tile's start=True clear wipes them).
# They are dependency-free, so they fill PE idle while the elementwise
# engines run, raising the streaming duty cycle above the HAM activity
# threshold so the clock stays at 2.4 GHz instead of oscillating.
HEAT_N = int(os.environ.get("ATTN_HEAT_N", "2"))
HEAT_EVERY = int(os.environ.get("ATTN_HEAT_EVERY", "1"))
HEAT_W = 0  # old stp-bank heater, measured slower; kept out
I16 = mybir.dt.int16


def build_nc():
    from contextlib import ExitStack

    nc = bacc.Bacc()
    # host-prepped inputs (bf16):
    #   q2: Q^T duplicated into both partition halves      [BL, 128, T]
    #   k2: K^T, even chunks rows 0:64, odd rows 64:128    [BL, 128, T/2]
    #   v:  V with ones column                             [BL, T, D+1]
    q2_d = nc.dram_tensor("q2", [BL, P, T], BF16, kind="ExternalInput")
    k2_d = nc.dram_tensor("k2", [BL, P, T // 2], BF16, kind="ExternalInput")
    v_d = nc.dram_tensor("v", [BL, T, D + 1], BF16, kind="ExternalInput")
    o_d = nc.dram_tensor("o", [BL, T, D], F32, kind="ExternalOutput")

    with tile.TileContext(nc) as tc, ExitStack() as ctx:
        singles = ctx.enter_context(tc.tile_pool(name="singles", bufs=1))
        wpool = ctx.enter_context(tc.tile_pool(name="wts", bufs=4))
        pepool = ctx.enter_context(tc.tile_pool(name="pexp", bufs=8))
        oout_pool = ctx.enter_context(tc.tile_pool(name="oout", bufs=3))
        rec_pool = ctx.enter_context(tc.tile_pool(name="rec", bufs=8))
        st_ps = ctx.enter_context(tc.tile_pool(name="stps", bufs=3, space="PSUM"))
        op_ps = ctx.enter_context(tc.tile_pool(name="opps", bufs=2, space="PSUM"))

        ebias = singles.tile([P, 1], F32)
        nc.vector.memset(ebias, EBIAS)
        # precomputed 0/1 causal triangle mask (keep where f >= p), applied
        # by DVE multiplies to the diagonal 128x128 blocks only
        tri0f = singles.tile([P, P], F32)
        nc.vector.memset(tri0f, 1.0)
        nc.gpsimd.affine_select(
            out=tri0f, in_=tri0f, compare_op=mybir.AluOpType.is_ge, fill=0.0,
            base=0, channel_multiplier=-1, pattern=[[1, P]],
        )
        tri0 = singles.tile([P, P], BF16)
        nc.vector.tensor_copy(out=tri0, in_=tri0f)

        wsrc = singles.tile([P, QW], BF16)
        nc.vector.memset(wsrc, 0.5)
        if PREWARM:
            # dense bf16 matmul burst on dummy data, scheduled during the
            # initial input-DMA stall (no data deps): holds the PE busy for
            # >3.4us so the HAM clock gate opens to 2.4 GHz before the real
            # stream starts. Uses an "st" pool slot (released before the
            # third S^T pair needs it) -> no extra PSUM bank.
            wps = st_ps.tile([P, 2, QW], F32, tag="st", name="warm")
            for _ in range(PREWARM_N):
                nc.tensor.matmul(
                    out=wps[:, 0, :], lhsT=wsrc[:, 0:P], rhs=wsrc,
                    start=True, stop=True,
                )

        def load_batch(b):
            # split the q/k loads so q-tile 0 (pairs 0-1) can start as soon
            # as the small head slices land, instead of waiting for the
            # full-width transfers
            qt = wpool.tile([P, T], BF16, tag="qt", name=f"qt{b}")
            nc.sync.dma_start(out=qt[:, 0:QW], in_=q2_d[b, :, 0:QW])
            kt = wpool.tile([P, T // 2], BF16, tag="kt", name=f"kt{b}")
            nc.sync.dma_start(out=kt[:, 0 : 2 * P], in_=k2_d[b, :, 0 : 2 * P])
            vv = wpool.tile([P, NCH, D + 1], BF16, tag="vv", name=f"vv{b}")
            vsrc = v_d[b].rearrange("(c p) d -> p c d", p=P)
            nc.sync.dma_start(out=vv, in_=vsrc)
            nc.sync.dma_start(out=qt[:, QW:T], in_=q2_d[b, :, QW:T])
            nc.sync.dma_start(out=kt[:, 2 * P :], in_=k2_d[b, :, 2 * P :])
            return qt, kt, vv

        def compute_qtile(b, i, qt, kt, vv):
            # O accumulator: [q_sub 128, s, d+1] - 4 subchunks in ONE PSUM
            # bank; the single start=True matmul clears the whole bank.
            # Two spare rows (NSUB, NSUB+1) are the HAM-keeper scratch.
            op = op_ps.tile([P, NSUB + 2, D + 1], F32, tag="op", name=f"op{b}_{i}")
            # off-diagonal lead (shortest chain to the start=True PV), then
            # the diagonal pairs so their mask latency hides under the
            # remaining off-diagonal pipeline
            if i == 0:
                order = [0, 1]
            else:
                order = [0, 2 * i, 2 * i + 1] + list(range(1, 2 * i))
            first = True
            # (pair u, h) -> list of (subchunk s, mask) PV jobs; mask is
            # None (full block), tri0 (diagonal block), or "skip" handled
            # by omission.  chunk index c = 2u + h covers keys
            # [c*128, (c+1)*128); subchunk s covers queries
            # [i*512 + s*128, +128) -> fully masked iff c > 4i + s,
            # diagonal iff c == 4i + s.
            jobs = {}
            for oidx, u in enumerate(order):
                for h in range(2):
                    c = 2 * u + h
                    lst = []
                    for s in range(NSUB):
                        if c > 4 * i + s:
                            continue                      # fully masked
                        lst.append((s, tri0 if c == 4 * i + s else None))
                    jobs[(u, h)] = lst
            last_u = order[-1]
            for oidx, u in enumerate(order):
                half = u == 2 * i + 1                     # outer diagonal pair
                w = 256 if half else QW                   # live q-width
                lo = QW - w                               # first live q col
                stp = st_ps.tile([P, 2, QW], F32, tag="st", name=f"st{b}_{i}_{u}")
                pexp = pepool.tile([P, 2, QW], BF16, tag="pe", name=f"pe{b}_{i}_{u}")
                for h in range(2):
                    # concurrent row-packed matmuls target DIFFERENT PSUM
                    # banks (stp[:, h] is bank h of the tile)
                    nc.tensor.matmul(
                        out=stp[:, h, lo : lo + w],
                        lhsT=kt[h * D : (h + 1) * D, u * P : (u + 1) * P],
                        rhs=qt[h * D : (h + 1) * D, i * QW + lo : (i + 1) * QW],
                        start=True,
                        stop=True,
                    )
                if DVE_EXP and u < 2 * i and u % 2 == 1:
                    # Schraudolph bf16 exp on the DVE: one fused
                    # multiply-add straight into the bf16 bit pattern
                    nc.vector.tensor_scalar(
                        out=pexp[:, :, lo : lo + w].bitcast(I16),
                        in0=stp[:, :, lo : lo + w],
                        scalar1=SCH_A,
                        scalar2=SCH_B,
                        op0=mybir.AluOpType.mult,
                        op1=mybir.AluOpType.add,
                    )
                else:
                    nc.scalar.activation(
                        out=pexp[:, :, lo : lo + w],
                        in_=stp[:, :, lo : lo + w],
                        func=mybir.ActivationFunctionType.Exp,
                        bias=ebias,
                        scale=SCALE,
                    )
                if HEAT_N and oidx % HEAT_EVERY == 0:
                    # HAM keeper (see top): garbage accumulate into the op
                    # bank's spare rows; ready immediately, fills PE idle
                    for r in range(HEAT_N):
                        nc.tensor.matmul(
                            out=op[:, NSUB + (r % 2), :],
                            lhsT=wsrc[:, 0:P],
                            rhs=wsrc[:, 0 : D + 1],
                            start=False,
                            stop=False,
                            skip_group_check=True,
                        )
                # triangle masks on the diagonal 128-blocks (GPSIMD; it is
                # otherwise idle)
                meng = nc.gpsimd if POOL_MASK else nc.vector
                for h in range(2):
                    for s, msk in jobs[(u, h)]:
                        if msk is not None:
                            blk = pexp[:, h, s * P : (s + 1) * P]
                            meng.tensor_mul(out=blk, in0=blk, in1=msk)
                # PV, direct-O: unmasked subchunks first (depend only on
                # exp), masked ones last (wait for the DVE mask)
                seq = []
                for h in range(2):
                    seq += [(h, s) for s, m in jobs[(u, h)] if m is None]
                for h in range(2):
                    seq += [(h, s) for s, m in jobs[(u, h)] if m is not None]
                for n, (h, s) in enumerate(seq):
                    stop = u == last_u and n == len(seq) - 1
                    nc.tensor.matmul(
                        out=op[:, s, :],
                        lhsT=pexp[:, h, s * P : (s + 1) * P],
                        rhs=vv[:, 2 * u + h, :],
                        start=first,
                        stop=stop,
                    )
                    first = False
            # epilogue: normalize by the sums column, straight from PSUM;
            # one reciprocal + one broadcast multiply for all 4 subchunks
            rec = rec_pool.tile([P, NSUB], F32, tag="rec", name=f"rec{b}_{i}")
            nc.vector.reciprocal(out=rec, in_=op[:, 0:NSUB, D])
            oout = oout_pool.tile([P, NSUB, D], F32, tag="oo", name=f"oo{b}_{i}")
            nc.vector.tensor_mul(
                out=oout,
                in0=op[:, 0:NSUB, 0:D],
                in1=rec.unsqueeze(2).broadcast_to([P, NSUB, D]),
            )
            nc.sync.dma_start(
                out=o_d[b, i * QW : (i + 1) * QW, :].rearrange(
                    "(m p) d -> p m d", p=P
                ),
                in_=oout,
            )

        for b in range(BL):
            qt, kt, vv = load_batch(b)
            for i in range(NQT):
                compute_qtile(b, i, qt, kt, vv)

    return nc


_NC_CACHE = None


def _get_nc():
    global _NC_CACHE
    if _NC_CACHE is None:
        nc = build_nc()
        nc.finalize()
        _NC_CACHE = nc
    return _NC_CACHE


def prep_inputs(queries, keys, values):
    """Host-side shard + layout prep (numpy only)."""
    q = np.asarray(queries, dtype=np.float32)
    k = np.asarray(keys, dtype=np.float32)
    v = np.asarray(values, dtype=np.float32)
    assert q.shape == (B, T, D), q.shape
    qT = q.transpose(0, 2, 1)                                  # [B, 64, T]
    q2 = np.concatenate([qT, qT], axis=1)                      # [B, 128, T]
    kT = k.transpose(0, 2, 1).reshape(B, D, NCH, P)            # [B, 64, 16, 128]
    k2 = np.concatenate(
        [
            kT[:, :, 0::2, :].reshape(B, D, T // 2),
            kT[:, :, 1::2, :].reshape(B, D, T // 2),
        ],
        axis=1,
    )                                                          # [B, 128, T/2]
    va = np.concatenate([v, np.ones((B, T, 1), np.float32)], axis=-1)
    q2 = np.ascontiguousarray(q2.astype(ml_dtypes.bfloat16))
    k2 = np.ascontiguousarray(k2.astype(ml_dtypes.bfloat16))
    va = np.ascontiguousarray(va.astype(ml_dtypes.bfloat16))
    return [
        {
            "q2": q2[c * BL : (c + 1) * BL],
            "k2": k2[c * BL : (c + 1) * BL],
            "v": va[c * BL : (c + 1) * BL],
        }
        for c in range(NCORES)
    ]


def run(queries, keys, values, trace=False):
    nc = _get_nc()
    core_ids = list(range(NCORES))
    in_maps = prep_inputs(queries, keys, values)
    try:
        res = run_bass_kernel_spmd(nc, in_maps, core_ids, trace=trace)
    except Exception:
        # transient NRT_EXEC_UNIT_UNRECOVERABLE has been observed once in
        # ~30 runs; a straight retry recovers
        res = run_bass_kernel_spmd(nc, in_maps, core_ids, trace=trace)
    out = np.concatenate([res.results[c]["o"] for c in core_ids], axis=0)
    return out.astype(np.float32), res


def kernel(queries, keys, values):
    out, _ = run(queries, keys, values, trace=False)
    return out
